# revision 49
# baseline (speedup 1.0000x reference)
"""GatedAttention TRN2 kernel — 8-core tensor-parallel (1 kv-head group per core).

Self-contained: host-side shard/layout prep + Bass/Tile kernel + gather.

Per-core dataflow (all device tensors feature-on-partition, "T" layouts):
  qkvT = W_c.T @ xT           (bf16 matmuls, PSUM accumulation, FWL-friendly
  128-col stationaries; per-head gate columns at partitions 0/32/64/96)
  q-RMS scale via block-diag selector matmul (the M=128 selector both sums
  squares per head and broadcasts the sum to all 64 head rows for free);
  Newton-rsqrt on [128,512] tiles split across DVE (p=0) and gpsimd (p=1)
  RoPE in bf16 on DVE (2x rate) with host-prefolded bf16 cos/sin tables
  scoresT[sj,si] per head, row-quadrant head pairs on the PE array
  exp on ACT with per-partition scale = 0.125 * rsqrt(mean k^2)
  P@V with V padded to M=128 (ones col 64 -> fused softmax denominators)
  u = (1+exp(-gate))*den read per-head straight from PSUM row 64 into a
  row-packed [128,512] tile; one packed Newton reciprocal chain; per-head
  broadcast back to 64 rows via a selector matmul on the PE
  out_partial = attnT_scaled.T @ Wo_c ; host sums the 8 partials.

Software pipeline: per block B the emission order is rms-matmuls(B),
v-transpose(B), QKV(B+1), rope(B) (DVE/gpsimd only, hidden under QKV(B+1)
on the PE), x-prefetch(B+2), outproj(B-1), attention(B), extract(B+1).
All bulk DMA (weights, tables, x tiles, output stores) runs on the gpsimd
SWDGE ring which spreads descriptors over the 14-queue pool; the 2-queue
sync ring carries only tiny constants.  Engine-SBUF access patterns must
start at partitions 0/32/64/96; cross-partition data movement is DVE-only
(gpsimd Q7 cores cannot shuffle across their 16-partition slices).
"""
import math
import os
import sys
import numpy as np
import ml_dtypes

BF16 = ml_dtypes.bfloat16

H, NH, KVH, HD = 2048, 32, 8, 64
G = NH // KVH          # 4 q heads per core
S = 2048
EPS = 1e-6
THETA = 1000000.0
SCALE = 1.0 / math.sqrt(HD)
NCORES = 8
HC = H // 128          # 16 h-chunks
NB = S // 512          # 4 si-blocks
NJ = S // 128          # 16 sj-chunks

_BUILT = {}
LAST_EXEC_NS = None


# ---------------------------------------------------------------- host prep
def _host_prep(hidden_states, Wq, Wk, Wv, Wo, g_q, g_k):
    x = np.ascontiguousarray(np.asarray(hidden_states, np.float32).reshape(S, H))
    Wq = np.asarray(Wq, np.float32)
    Wk = np.asarray(Wk, np.float32)
    Wv = np.asarray(Wv, np.float32)
    Wo = np.asarray(Wo, np.float32)
    g_q = np.asarray(g_q, np.float32)
    g_k = np.asarray(g_k, np.float32)

    xT = np.ascontiguousarray(x.T).astype(BF16)

    inv_freq = 1.0 / (THETA ** (np.arange(0, HD, 2, dtype=np.float32) / HD))
    pos = np.arange(S, dtype=np.float32)
    emb = np.concatenate([pos[:, None] * inv_freq[None, :]] * 2, axis=-1)  # [S,64]
    cos = np.cos(emb).T.astype(np.float32)   # [64, S]
    sin = np.sin(emb).T.astype(np.float32)
    sign = np.where(np.arange(HD) < HD // 2, -1.0, 1.0).astype(np.float32)[:, None]
    cosq = np.ascontiguousarray(cos * g_q[:, None]).astype(BF16)
    sinq = np.ascontiguousarray(sin * sign * np.roll(g_q, -32)[:, None]).astype(BF16)
    cosk = np.ascontiguousarray(cos * g_k[:, None]).astype(BF16)
    sink = np.ascontiguousarray(sin * sign * np.roll(g_k, -32)[:, None]).astype(BF16)

    in_maps = []
    for c in range(NCORES):
        Wq_g = Wq[:, c * (G * HD + G):(c + 1) * (G * HD + G)]
        gpad = np.zeros((H, 128), np.float32)
        for p in range(2):
            for hh in range(2):
                # gate for head (p,hh) lands on PSUM partition 64p+32hh — a
                # legal SBUF/PSUM access-start for the per-head exp reads
                gpad[:, 64 * p + 32 * hh] = Wq_g[:, G * HD + 2 * p + hh]
        W_c = np.ascontiguousarray(np.concatenate(
            [Wq_g[:, :G * HD],
             Wk[:, c * HD:(c + 1) * HD],
             Wv[:, c * HD:(c + 1) * HD],
             gpad], axis=1))                                   # [H, 512]
        Wo_c = np.ascontiguousarray(Wo[c * G * HD:(c + 1) * G * HD, :])  # [256,H]
        in_maps.append({"xT": xT, "W": W_c.astype(BF16), "Wo": Wo_c.astype(BF16),
                        "cosq": cosq, "sinq": sinq, "cosk": cosk, "sink": sink})
    return in_maps


# ---------------------------------------------------------------- bass build
def _build_nc():
    import concourse.bass as bass
    import concourse.mybir as mybir
    import concourse.tile as tile
    from concourse import bacc
    from concourse.masks import make_identity, make_upper_triangular

    dt = mybir.dt
    f32 = dt.float32
    bf16 = dt.bfloat16
    AF = mybir.ActivationFunctionType

    nc = bacc.Bacc("TRN2", target_bir_lowering=False, debug=False,
                   num_devices=NCORES)

    xT_d = nc.dram_tensor("xT", [H, S], bf16, kind="ExternalInput")
    W_d = nc.dram_tensor("W", [H, 512], bf16, kind="ExternalInput")
    Wo_d = nc.dram_tensor("Wo", [G * HD, H], bf16, kind="ExternalInput")
    cosq_d = nc.dram_tensor("cosq", [HD, S], bf16, kind="ExternalInput")
    sinq_d = nc.dram_tensor("sinq", [HD, S], bf16, kind="ExternalInput")
    cosk_d = nc.dram_tensor("cosk", [HD, S], bf16, kind="ExternalInput")
    sink_d = nc.dram_tensor("sink", [HD, S], bf16, kind="ExternalInput")
    out_d = nc.dram_tensor("out", [S, H], f32, kind="ExternalOutput")

    import contextlib
    with tile.TileContext(nc) as tc, contextlib.ExitStack() as ctx:
        const = ctx.enter_context(tc.tile_pool(name="const", bufs=1))
        big = ctx.enter_context(tc.tile_pool(name="big", bufs=1))
        xpool = ctx.enter_context(tc.tile_pool(name="xp", bufs=32))
        rawp = ctx.enter_context(tc.tile_pool(name="raw", bufs=2))
        tmpp = ctx.enter_context(tc.tile_pool(name="tmp", bufs=2))
        sqp = ctx.enter_context(tc.tile_pool(name="sq", bufs=2))
        rnp = ctx.enter_context(tc.tile_pool(name="rn", bufs=2))
        bcp = ctx.enter_context(tc.tile_pool(name="bc", bufs=2))
        expp = ctx.enter_context(tc.tile_pool(name="expp", bufs=4))
        outs = ctx.enter_context(tc.tile_pool(name="outs", bufs=3))
        smal = ctx.enter_context(tc.tile_pool(name="smal", bufs=2))
        psum = ctx.enter_context(tc.tile_pool(name="ps", bufs=1, space="PSUM"))

        # ---------------- constants (id64/tri are built on gpsimd, so they
        # are emitted after the critical startup loads — see schedule)
        id64 = const.tile([64, 64], bf16, tag="id64")
        tri = const.tile([128, 128], bf16, tag="tri")
        ones = const.tile([128, 1], bf16, tag="ones")
        nc.vector.memset(ones, 1.0)
        # block-diagonal selector: sums 64-row head blocks AND broadcasts the
        # result back to all 64 rows of the head (out partition p gets the sum
        # over contraction rows of the same head).
        esel2 = const.tile([128, 128], bf16, tag="esel2")
        nc.vector.memset(esel2, 0.0)
        nc.vector.memset(esel2[0:64, 0:64], 1.0)
        nc.vector.memset(esel2[64:128, 64:128], 1.0)
        # per-head scale row broadcast: sel_p[c,m]=1 where source row c feeds
        # head rows m (rows 0/32/64/96 are legal memset partition starts)
        selp = [const.tile([128, 128], f32, tag=f"sel{p}",
                name=f"sel{p}") for p in range(2)]
        for p in range(2):
            nc.vector.memset(selp[p], 0.0)
            nc.vector.memset(selp[p][64 * p:64 * p + 1, 0:64], 1.0)
            nc.vector.memset(selp[p][64 * p + 32:64 * p + 33, 64:128], 1.0)
        SIGMA = 0.0430
        EXPBIT_SCALE = math.log(2.0) / (1 << 23)
        b_rsq = const.tile([128, 1], f32, tag="brsq")
        nc.vector.memset(b_rsq, 0.5 * math.log(2.0) * (127 + SIGMA + 6))
        b_rcp = const.tile([128, 1], f32, tag="brcp")
        nc.vector.memset(b_rcp, math.log(2.0) * (127 + SIGMA))
        u32 = dt.uint32

        # ---------------- persistent activations
        kk2 = big.tile([128, S], bf16, tag="kk2")
        v_sb = big.tile([128, NJ, 128], bf16, tag="v")
        nc.vector.memset(v_sb, 0.0)
        nc.vector.memset(v_sb[:, :, 64:65], 1.0)
        rkT_sb = big.tile([128, NJ], f32, tag="rkT")

        # xt prefetch: one si-block = 16 [128,512] chunks; keep 2 blocks in
        # flight (tag bufs=32).
        xts = {}

        def load_x(b):
            sp = slice(b * 512, (b + 1) * 512)
            ts = []
            for hc in range(HC):
                xt = xpool.tile([128, 512], bf16, tag="xt", bufs=32,
                                name=f"xt{b}_{hc}")
                nc.gpsimd.dma_start(out=xt, in_=xT_d[hc * 128:(hc + 1) * 128, sp])
                ts.append(xt)
            xts[b] = ts

        # ---------------- resident weights / tables
        # Everything on the gpsimd ring (spreads transfers over the 14-queue
        # pool).  Per-hc W tiles interleaved with xt(0) chunks so QKV(0)
        # matmul hc can start as soon as its own W/x chunks land; tables and
        # Wo are emitted after the critical-path loads (needed later).
        W_hc = [big.tile([128, 512], bf16, tag=f"W{hc}", name=f"W{hc}")
                for hc in range(HC)]

        def load_w_x0():
            sp = slice(0, 512)
            ts = []
            for hc in range(HC):
                nc.gpsimd.dma_start(out=W_hc[hc],
                                    in_=W_d[hc * 128:(hc + 1) * 128, :])
                xt = xpool.tile([128, 512], bf16, tag="xt", bufs=32,
                                name=f"xt0_{hc}")
                nc.gpsimd.dma_start(out=xt, in_=xT_d[hc * 128:(hc + 1) * 128, sp])
                ts.append(xt)
            xts[0] = ts

        def load_tables():
            def pair_table(src_d, tag):
                t = big.tile([128, S], bf16, tag=tag, name=tag)
                src = src_d.ap()
                ap2 = bass.AP(tensor=src.tensor, offset=src.offset,
                              ap=[[0, 2]] + list(src.ap))
                nc.gpsimd.dma_start(out=t, in_=ap2)
                return t

            cosq_sb = pair_table(cosq_d, "cosq")
            cosk_sb = big.tile([64, S], bf16, tag="cosk")
            nc.gpsimd.dma_start(out=cosk_sb, in_=cosk_d[:, :])
            sinq_sb = pair_table(sinq_d, "sinq")
            sink_sb = big.tile([64, S], bf16, tag="sink")
            nc.gpsimd.dma_start(out=sink_sb, in_=sink_d[:, :])
            return cosq_sb, sinq_sb, cosk_sb, sink_sb

        # per-block state handed across pipeline stages
        ps_cc_s, ps_g_s = {}, {}
        qr_s, kr_s, vr_s, sig_s = {}, {}, {}, {}

        def qkv(b):
            """QKV projection matmuls for si-block b (PE only)."""
            ps_cc = [psum.tile([128, 512], f32, tag="qkv3", bufs=3,
                               name=f"pscc{b}_{cc}") for cc in range(3)]
            ps_g = psum.tile([128, 512], f32, tag="gate", bufs=1,
                             name=f"psg{b}")
            for hc in range(HC):
                xt = xts[b][hc]
                st = (hc == 0)
                fin = (hc == HC - 1)
                for cc in range(3):
                    nc.tensor.matmul(ps_cc[cc][:],
                                     W_hc[hc][:, cc * 128:(cc + 1) * 128],
                                     xt, start=st, stop=fin)
                nc.tensor.matmul(ps_g[:], W_hc[hc][:, 384:512], xt,
                                 start=st, stop=fin)
            ps_cc_s[b] = ps_cc
            ps_g_s[b] = ps_g

        def extract(b):
            """Pull QKV(b) out of PSUM (ACT copies + gate exps)."""
            ps_cc, ps_g = ps_cc_s[b], ps_g_s[b]
            qr = [rawp.tile([128, 512], bf16, tag=f"qr{p}", name=f"qr{b}_{p}")
                  for p in range(2)]
            kr = rawp.tile([64, 512], bf16, tag="kr", name=f"kr{b}")
            vr = rawp.tile([64, 512], bf16, tag="vr", name=f"vr{b}")
            for p in range(2):
                nc.scalar.copy(qr[p], ps_cc[p][:])
            nc.scalar.copy(kr, ps_cc[2][0:64, :])
            nc.scalar.copy(vr, ps_cc[2][64:128, :])
            # exp(-gate) per head, packed at rows 0/32/64/96 (legal SBUF
            # partition starts) of one tile; PSUM row slices are exempt from
            # the partition-start rule so ps_g can be read per-head.
            sig_q = smal.tile([128, 512], f32, tag="sig", bufs=2,
                              name=f"eg{b}")
            for p in range(2):
                for hh in range(2):
                    r = 64 * p + 32 * hh
                    nc.scalar.activation(sig_q[r:r + 1, :],
                                         ps_g[r:r + 1, :],
                                         AF.Exp, scale=-1.0)
            qr_s[b], kr_s[b], vr_s[b], sig_s[b] = qr, kr, vr, sig_q

        rms_st = {}

        def rms_phase1(b):
            """RMS-scale matmuls + Newton seeds + k-side chain.  Emitted
            before QKV(b+1) so the Newton/RoPE chains (phase 2) can run on
            DVE/gpsimd while the PE crunches the next projection."""
            qr, kr = qr_s[b], kr_s[b]

            sqs = [None, None]
            pss = [None, None]
            y0s = [None, None]
            for p in range(2):
                eng = nc.vector if p == 0 else nc.gpsimd
                sq = sqp.tile([128, 512], bf16, tag=f"sq{p}", name=f"sq{b}_{p}")
                eng.tensor_mul(sq, qr[p], qr[p])
                sqs[p] = sq
            ksq = sqp.tile([64, 512], bf16, tag="ksq", name=f"ksq{b}")
            nc.vector.tensor_mul(ksq, kr, kr)
            for p in range(2):
                ps_rq = psum.tile([128, 512], f32, tag="sc", bufs=2,
                                  name=f"psrq{b}_{p}")
                nc.tensor.matmul(ps_rq[:], esel2, sqs[p], start=True, stop=True)
                pss[p] = ps_rq
            ps_rk = psum.tile([128, 4], f32, tag="sc", bufs=2, name=f"psrk{b}")
            for j in range(4):
                nc.tensor.matmul(ps_rk[:, j:j + 1],
                                 ksq[:, j * 128:(j + 1) * 128],
                                 ones[0:64, :], start=True, stop=True)
            a_t = [None, None]
            for p in range(2):
                y0 = rnp.tile([128, 512], f32, tag=f"rnB{p}", name=f"y0{b}_{p}")
                nc.scalar.activation(y0, pss[p][:].bitcast(u32), AF.Exp,
                                     bias=b_rsq, scale=-0.5 * EXPBIT_SCALE)
                y0s[p] = y0
                if p == 0:
                    a_t[p] = pss[p]
                else:
                    # gpsimd cannot read PSUM; stage the sums in SBUF
                    a_t[p] = rnp.tile([128, 512], f32, tag="rqs",
                                      name=f"rqs{b}")
                    nc.scalar.copy(a_t[p], pss[p][:])

            # k-side Newton ([128,4] ops are ~free) -> rkT ready early for
            # the attention exps
            yk = smal.tile([128, 4], f32, tag="smB", name=f"yk{b}")
            nc.scalar.activation(yk, ps_rk[:].bitcast(u32), AF.Exp,
                                 bias=b_rsq, scale=-0.5 * EXPBIT_SCALE)
            for it in range(2):
                last = (it == 1)
                tk = smal.tile([128, 4], f32, tag="smA", name=f"tk{b}")
                nc.vector.tensor_mul(tk, ps_rk[:], yk)
                nc.vector.tensor_mul(tk, tk, yk)
                nc.vector.tensor_scalar(tk, tk,
                                        (-0.5 * SCALE / HD) if last else (-0.5 / HD),
                                        (1.5 * SCALE) if last else 1.5,
                                        mybir.AluOpType.mult, mybir.AluOpType.add)
                if last:
                    nc.vector.tensor_mul(rkT_sb[:, b * 4:(b + 1) * 4], yk, tk)
                else:
                    ykn = smal.tile([128, 4], f32, tag="smB", name=f"ykn{b}")
                    nc.vector.tensor_mul(ykn, yk, tk)
                    yk = ykn
            rms_st[b] = (a_t, y0s)

        def rope_phase2(b):
            """Newton iterations + RoPE -> qf, kk2.  DVE handles the p=0
            chain and k; gpsimd handles the p=1 Newton and all rotate-half
            copies.  No PE work here — runs concurrently with QKV(b+1)."""
            sp = slice(b * 512, (b + 1) * 512)
            qr, kr = qr_s[b], kr_s[b]
            a_t, y0s = rms_st.pop(b)

            # rotate-half staging (cross-partition copies: DVE only — gpsimd
            # Q7 cores are hardwired to their own 16-partition slice and
            # cannot shuffle across partitions)
            qss = [None, None]
            for p in range(2):
                qs = tmpp.tile([128, 512], bf16, tag=f"qs{p}",
                               name=f"qs{b}_{p}")
                for g in range(2):
                    bb = g * 64
                    nc.vector.tensor_copy(qs[bb:bb + 32, :],
                                          qr[p][bb + 32:bb + 64, :])
                    nc.vector.tensor_copy(qs[bb + 32:bb + 64, :],
                                          qr[p][bb:bb + 32, :])
                qss[p] = qs
            ks = tmpp.tile([64, 512], bf16, tag="ks", name=f"ks{b}")
            nc.vector.tensor_copy(ks[0:32, :], kr[32:64, :])
            nc.vector.tensor_copy(ks[32:64, :], kr[0:32, :])

            rq = [None, None]
            for p in range(2):
                eng = nc.vector if p == 0 else nc.gpsimd
                y0 = y0s[p]
                for it in range(1):
                    tn = rnp.tile([128, 512], f32, tag=f"rnA{p}",
                                  name=f"tn{b}_{p}")
                    eng.tensor_mul(tn, a_t[p][:], y0)
                    eng.tensor_mul(tn, tn, y0)
                    eng.tensor_scalar(tn, tn, -0.5 / HD, 1.5,
                                      mybir.AluOpType.mult,
                                      mybir.AluOpType.add)
                    yn = rnp.tile([128, 512], f32, tag=f"rnB{p}",
                                  name=f"yn{b}_{p}")
                    eng.tensor_mul(yn, y0, tn)
                    y0 = yn
                rq[p] = y0

            qf = [rawp.tile([128, 512], bf16, tag=f"qf{p}", name=f"qf{b}_{p}",
                            bufs=2) for p in range(2)]

            def rope_q(p):
                t1 = tmpp.tile([128, 512], bf16, tag="t1")
                nc.vector.tensor_mul(t1, qr[p], cosq_sb[:, sp])
                t2 = tmpp.tile([128, 512], bf16, tag="t2")
                nc.vector.tensor_mul(t2, qss[p], sinq_sb[:, sp])
                nc.vector.tensor_add(t2, t1, t2)
                nc.vector.tensor_mul(qf[p], t2, rq[p])

            # p=0 q first (needed by the first scores), then k (needed by
            # every scores), then p=1 q (needed only mid-attention)
            rope_q(0)
            t1k = tmpp.tile([64, 512], bf16, tag="t1")
            nc.vector.tensor_mul(t1k, kr, cosk_sb[:, sp])
            t2k = tmpp.tile([64, 512], bf16, tag="t2")
            nc.vector.tensor_mul(t2k, ks, sink_sb[:, sp])
            nc.vector.tensor_add(kk2[0:64, sp], t1k, t2k)
            nc.vector.tensor_copy(kk2[64:128, sp], kk2[0:64, sp])
            rope_q(1)
            return qf

        def vtransp(b):
            vr = vr_s[b]
            for j in range(4):
                J = b * 4 + j
                ps_v = psum.tile([128, 64], bf16, tag="sc", bufs=2,
                                 name=f"psv{b}_{j}")
                nc.tensor.transpose(ps_v[:], vr[:, j * 128:(j + 1) * 128], id64)
                nc.scalar.copy(v_sb[:, J, 0:64], ps_v[:])

        def attention(b, qf):
            B = b
            sig_q = sig_s[b]
            at = [rawp.tile([128, 512], bf16, tag=f"at{p}", name=f"at{b}_{p}",
                            bufs=2) for p in range(2)]
            attcp = [None, None]
            u_q = smal.tile([128, 512], f32, tag="u", bufs=2, name=f"u{b}")
            nc.vector.memset(u_q, 1.0)

            for p in range(2):
                ps_att = [psum.tile([128, 512], f32, tag="att", bufs=2,
                                    name=f"psatt{b}_{p}_{hh}") for hh in range(2)]
                for J in range(4 * B + 4):
                    off = max(0, (J - 4 * B) * 128)
                    ex = []
                    for hh in range(2):
                        rb = hh * 64
                        ps_s = psum.tile([128, 512], f32, tag="sc", bufs=2,
                                         name="pss")
                        nc.tensor.matmul(
                            ps_s[:, off:512],
                            kk2[rb:rb + 64, J * 128:(J + 1) * 128],
                            qf[p][rb:rb + 64, off:512],
                            start=True, stop=True,
                            tile_position=(rb, 0))
                        et = expp.tile([128, 512], bf16, tag="expT", bufs=5,
                                       name="et")
                        nc.scalar.activation(et[:, off:512], ps_s[:, off:512],
                                             AF.Exp, scale=rkT_sb[:, J:J + 1])
                        if off > 0 or J == 4 * B:
                            nc.vector.tensor_mul(et[:, off:off + 128],
                                                 et[:, off:off + 128], tri)
                        ex.append(et)
                    for hh in range(2):
                        nc.tensor.matmul(
                            ps_att[hh][:, off:512],
                            v_sb[:, J, :],
                            ex[hh][:, off:512],
                            start=(J == 0), stop=(J == 4 * B + 3))

                # drain ps_att immediately: PV values to SBUF (ACT), and the
                # scale chain u=(1+exp(-gate))*den -> s=1/u (DVE, direct PSUM
                # row reads + fast reciprocal).  For p=0 this hides entirely
                # under the p=1 J-loop.
                acp = bcp.tile([128, 512], f32, tag="attcp", name=f"acp{b}_{p}")
                for hh in range(2):
                    r = 64 * p + 32 * hh
                    nc.scalar.copy(acp[64 * hh:64 * hh + 64, :],
                                   ps_att[hh][0:64, :])
                    nc.vector.scalar_tensor_tensor(u_q[r:r + 1, :],
                                                   sig_q[r:r + 1, :], 1.0,
                                                   ps_att[hh][64:65, :],
                                                   mybir.AluOpType.add,
                                                   mybir.AluOpType.mult)
                attcp[p] = acp

            # packed Newton reciprocal: all four denominators in one chain
            s_y = smal.tile([128, 512], f32, tag="sy", bufs=2, name=f"sy{b}")
            nc.scalar.activation(s_y, u_q[:].bitcast(u32), AF.Exp,
                                 bias=b_rcp, scale=-EXPBIT_SCALE)
            for it in range(2):  # cheap [128,512] ops; keep 2 for accuracy
                tu = smal.tile([128, 512], f32, tag="tu", bufs=2,
                               name=f"tu{b}")
                nc.vector.tensor_mul(tu, u_q, s_y)
                nc.vector.tensor_scalar(tu, tu, -1.0, 2.0,
                                        mybir.AluOpType.mult,
                                        mybir.AluOpType.add)
                s_n = smal.tile([128, 512], f32, tag="sy", bufs=2,
                                name=f"sn{b}")
                nc.vector.tensor_mul(s_n, s_y, tu)
                s_y = s_n

            # broadcast scales to head rows on the PE (two accumulated K=1
            # matmuls) and apply in one [128,512] multiply per head pair
            for p in range(2):
                sbc_ps = psum.tile([128, 512], f32, tag="sc", bufs=2,
                                   name=f"sbc{b}_{p}")
                nc.tensor.matmul(sbc_ps[:], selp[p], s_y, start=True,
                                 stop=True)
                nc.vector.tensor_mul(at[p], attcp[p], sbc_ps[:])
            return at

        def outproj(b, at):
            B = b
            for ss in range(4 * B, 4 * B + 4):
                ls = (ss - 4 * B) * 128
                for qtr in range(4):
                    ps_o = psum.tile([128, 512], f32, tag="qkv3", bufs=3,
                                     name="pso")
                    nc.tensor.matmul(ps_o[:], at[0][:, ls:ls + 128],
                                     Wo_sb[:, 0, qtr * 512:(qtr + 1) * 512],
                                     start=True, stop=False)
                    nc.tensor.matmul(ps_o[:], at[1][:, ls:ls + 128],
                                     Wo_sb[:, 1, qtr * 512:(qtr + 1) * 512],
                                     start=False, stop=True)
                    ot = outs.tile([128, 512], f32, tag="ot")
                    if qtr % 2 == 0:
                        nc.scalar.copy(ot, ps_o[:])
                    else:
                        nc.vector.tensor_copy(ot, ps_o[:])
                    nc.gpsimd.dma_start(
                        out=out_d[ss * 128:(ss + 1) * 128,
                                  qtr * 512:(qtr + 1) * 512],
                        in_=ot)

        # ---------------- pipelined schedule
        load_w_x0()
        make_identity(nc, id64)
        make_upper_triangular(nc, tri, val=1.0, diag=True)
        qkv(0)
        extract(0)
        load_x(1)
        cosq_sb, sinq_sb, cosk_sb, sink_sb = load_tables()
        Wo_sb = big.tile([128, 2, H], bf16, tag="Wo")
        nc.gpsimd.dma_start(out=Wo_sb, in_=Wo_d.ap().rearrange(
            "(cc p) h -> p cc h", p=128))
        at_prev = None
        for B in range(NB):
            rms_phase1(B)
            vtransp(B)
            if B + 1 < NB:
                qkv(B + 1)
            qf = rope_phase2(B)
            if B + 2 < NB:
                load_x(B + 2)
            if at_prev is not None:
                outproj(B - 1, at_prev)
            at_prev = attention(B, qf)
            if B + 1 < NB:
                extract(B + 1)
        outproj(NB - 1, at_prev)

    nc.compile()
    return nc


def _get_nc():
    if "nc" not in _BUILT:
        _BUILT["nc"] = _build_nc()
    return _BUILT["nc"]


# ---------------------------------------------------------------- entry point
def _install_ntff_hook():
    import types
    try:
        import antenv
        if "antenv.axon_hooks" in sys.modules:
            return True
        mod = types.ModuleType("antenv.axon_hooks")
        holder = [None]
        mod.set_axon_ntff_profile_hook = lambda h: holder.__setitem__(0, h)
        mod.get_axon_ntff_profile_hook = lambda: holder[0]
        sys.modules["antenv.axon_hooks"] = mod
        antenv.axon_hooks = mod
        from trn_agent_boot.trn_boot import _ntff_profile_via_ctypes
        hook = _ntff_profile_via_ctypes("/opt/axon/libaxon_pjrt.so")
        if hook is None:
            return False
        mod.set_axon_ntff_profile_hook(hook)
        return True
    except Exception:
        return False


def kernel(hidden_states, Wq, Wk, Wv, Wo, g_q, g_k):
    global LAST_EXEC_NS
    from concourse.bass_utils import run_bass_kernel_spmd

    in_maps = _host_prep(hidden_states, Wq, Wk, Wv, Wo, g_q, g_k)
    nc = _get_nc()
    trace = os.environ.get("KERNEL_TRACE", "0") == "1"
    if trace:
        trace = _install_ntff_hook()
    res = run_bass_kernel_spmd(nc, in_maps, list(range(NCORES)), trace=trace)
    LAST_EXEC_NS = res.exec_time_ns
    out = np.zeros((S, H), np.float32)
    for c in range(NCORES):
        out += res.results[c]["out"]
    return out.reshape(1, S, H).astype(np.float32)


# revision 50
# speedup vs baseline: 1.0131x; 1.0131x over previous
"""GatedAttention TRN2 kernel — 8-core tensor-parallel (1 kv-head group per core).

Self-contained: host-side shard/layout prep + Bass/Tile kernel + gather.

Per-core dataflow (all device tensors feature-on-partition, "T" layouts):
  qkvT = W_c.T @ xT           (bf16 matmuls, PSUM accumulation, FWL-friendly
  128-col stationaries; per-head gate columns at partitions 0/32/64/96)
  q-RMS scale via block-diag selector matmul (the M=128 selector both sums
  squares per head and broadcasts the sum to all 64 head rows for free);
  Newton-rsqrt on [128,512] tiles split across DVE (p=0) and gpsimd (p=1)
  RoPE in bf16 on DVE (2x rate) with host-prefolded bf16 cos/sin tables
  scoresT[sj,si] per head, row-quadrant head pairs on the PE array
  exp on ACT with per-partition scale = 0.125 * rsqrt(mean k^2)
  P@V with V padded to M=128 (ones col 64 -> fused softmax denominators)
  u = (1+exp(-gate))*den read per-head straight from PSUM row 64 into a
  row-packed [128,512] tile; one packed Newton reciprocal chain; per-head
  broadcast back to 64 rows via a selector matmul on the PE
  out_partial = attnT_scaled.T @ Wo_c ; host sums the 8 partials.

Software pipeline: per block B the emission order is rms-matmuls(B),
v-transpose(B), QKV(B+1), rope(B) (DVE/gpsimd only, hidden under QKV(B+1)
on the PE), x-prefetch(B+2), outproj(B-1), attention(B), extract(B+1).
All bulk DMA (weights, tables, x tiles, output stores) runs on the gpsimd
SWDGE ring which spreads descriptors over the 14-queue pool; the 2-queue
sync ring carries only tiny constants.  Engine-SBUF access patterns must
start at partitions 0/32/64/96; cross-partition data movement is DVE-only
(gpsimd Q7 cores cannot shuffle across their 16-partition slices).
"""
import math
import os
import sys
import numpy as np
import ml_dtypes

BF16 = ml_dtypes.bfloat16

H, NH, KVH, HD = 2048, 32, 8, 64
G = NH // KVH          # 4 q heads per core
S = 2048
EPS = 1e-6
THETA = 1000000.0
SCALE = 1.0 / math.sqrt(HD)
NCORES = 8
HC = H // 128          # 16 h-chunks
NB = S // 512          # 4 si-blocks
NJ = S // 128          # 16 sj-chunks

_BUILT = {}
LAST_EXEC_NS = None


# ---------------------------------------------------------------- host prep
def _host_prep(hidden_states, Wq, Wk, Wv, Wo, g_q, g_k):
    x = np.ascontiguousarray(np.asarray(hidden_states, np.float32).reshape(S, H))
    Wq = np.asarray(Wq, np.float32)
    Wk = np.asarray(Wk, np.float32)
    Wv = np.asarray(Wv, np.float32)
    Wo = np.asarray(Wo, np.float32)
    g_q = np.asarray(g_q, np.float32)
    g_k = np.asarray(g_k, np.float32)

    xT = np.ascontiguousarray(x.T).astype(BF16)

    inv_freq = 1.0 / (THETA ** (np.arange(0, HD, 2, dtype=np.float32) / HD))
    pos = np.arange(S, dtype=np.float32)
    emb = np.concatenate([pos[:, None] * inv_freq[None, :]] * 2, axis=-1)  # [S,64]
    cos = np.cos(emb).T.astype(np.float32)   # [64, S]
    sin = np.sin(emb).T.astype(np.float32)
    sign = np.where(np.arange(HD) < HD // 2, -1.0, 1.0).astype(np.float32)[:, None]
    cosq = np.ascontiguousarray(cos * g_q[:, None]).astype(BF16)
    sinq = np.ascontiguousarray(sin * sign * np.roll(g_q, -32)[:, None]).astype(BF16)
    cosk = np.ascontiguousarray(cos * g_k[:, None]).astype(BF16)
    sink = np.ascontiguousarray(sin * sign * np.roll(g_k, -32)[:, None]).astype(BF16)

    in_maps = []
    for c in range(NCORES):
        Wq_g = Wq[:, c * (G * HD + G):(c + 1) * (G * HD + G)]
        gpad = np.zeros((H, 128), np.float32)
        for p in range(2):
            for hh in range(2):
                # gate for head (p,hh) lands on PSUM partition 64p+32hh — a
                # legal SBUF/PSUM access-start for the per-head exp reads
                gpad[:, 64 * p + 32 * hh] = Wq_g[:, G * HD + 2 * p + hh]
        W_c = np.ascontiguousarray(np.concatenate(
            [Wq_g[:, :G * HD],
             Wk[:, c * HD:(c + 1) * HD],
             Wv[:, c * HD:(c + 1) * HD],
             gpad], axis=1))                                   # [H, 512]
        Wo_c = np.ascontiguousarray(Wo[c * G * HD:(c + 1) * G * HD, :])  # [256,H]
        in_maps.append({"xT": xT, "W": W_c.astype(BF16), "Wo": Wo_c.astype(BF16),
                        "cosq": cosq, "sinq": sinq, "cosk": cosk, "sink": sink})
    return in_maps


# ---------------------------------------------------------------- bass build
def _build_nc():
    import concourse.bass as bass
    import concourse.mybir as mybir
    import concourse.tile as tile
    from concourse import bacc
    from concourse.masks import make_identity, make_upper_triangular

    dt = mybir.dt
    f32 = dt.float32
    bf16 = dt.bfloat16
    AF = mybir.ActivationFunctionType

    nc = bacc.Bacc("TRN2", target_bir_lowering=False, debug=False,
                   num_devices=NCORES)

    xT_d = nc.dram_tensor("xT", [H, S], bf16, kind="ExternalInput")
    W_d = nc.dram_tensor("W", [H, 512], bf16, kind="ExternalInput")
    Wo_d = nc.dram_tensor("Wo", [G * HD, H], bf16, kind="ExternalInput")
    cosq_d = nc.dram_tensor("cosq", [HD, S], bf16, kind="ExternalInput")
    sinq_d = nc.dram_tensor("sinq", [HD, S], bf16, kind="ExternalInput")
    cosk_d = nc.dram_tensor("cosk", [HD, S], bf16, kind="ExternalInput")
    sink_d = nc.dram_tensor("sink", [HD, S], bf16, kind="ExternalInput")
    out_d = nc.dram_tensor("out", [S, H], f32, kind="ExternalOutput")

    import contextlib
    with tile.TileContext(nc) as tc, contextlib.ExitStack() as ctx:
        const = ctx.enter_context(tc.tile_pool(name="const", bufs=1))
        big = ctx.enter_context(tc.tile_pool(name="big", bufs=1))
        xpool = ctx.enter_context(tc.tile_pool(name="xp", bufs=32))
        rawp = ctx.enter_context(tc.tile_pool(name="raw", bufs=2))
        tmpp = ctx.enter_context(tc.tile_pool(name="tmp", bufs=2))
        sqp = ctx.enter_context(tc.tile_pool(name="sq", bufs=2))
        rnp = ctx.enter_context(tc.tile_pool(name="rn", bufs=2))
        bcp = ctx.enter_context(tc.tile_pool(name="bc", bufs=2))
        expp = ctx.enter_context(tc.tile_pool(name="expp", bufs=4))
        outs = ctx.enter_context(tc.tile_pool(name="outs", bufs=3))
        smal = ctx.enter_context(tc.tile_pool(name="smal", bufs=2))
        psum = ctx.enter_context(tc.tile_pool(name="ps", bufs=1, space="PSUM"))

        # ---------------- constants (id64/tri are built on gpsimd, so they
        # are emitted after the critical startup loads — see schedule)
        id64 = const.tile([64, 64], bf16, tag="id64")
        tri = const.tile([128, 128], bf16, tag="tri")
        ones = const.tile([128, 1], bf16, tag="ones")
        nc.vector.memset(ones, 1.0)
        # block-diagonal selector: sums 64-row head blocks AND broadcasts the
        # result back to all 64 rows of the head (out partition p gets the sum
        # over contraction rows of the same head).
        esel2 = const.tile([128, 128], bf16, tag="esel2")
        nc.vector.memset(esel2, 0.0)
        nc.vector.memset(esel2[0:64, 0:64], 1.0)
        nc.vector.memset(esel2[64:128, 64:128], 1.0)
        # per-head scale row broadcast: sel_p[c,m]=1 where source row c feeds
        # head rows m (rows 0/32/64/96 are legal memset partition starts)
        selp = [const.tile([128, 128], f32, tag=f"sel{p}",
                name=f"sel{p}") for p in range(2)]
        for p in range(2):
            nc.vector.memset(selp[p], 0.0)
            nc.vector.memset(selp[p][64 * p:64 * p + 1, 0:64], 1.0)
            nc.vector.memset(selp[p][64 * p + 32:64 * p + 33, 64:128], 1.0)
        SIGMA = 0.0430
        EXPBIT_SCALE = math.log(2.0) / (1 << 23)
        b_rsq = const.tile([128, 1], f32, tag="brsq")
        nc.vector.memset(b_rsq, 0.5 * math.log(2.0) * (127 + SIGMA + 6))
        b_rcp = const.tile([128, 1], f32, tag="brcp")
        nc.vector.memset(b_rcp, math.log(2.0) * (127 + SIGMA))
        u32 = dt.uint32

        # ---------------- persistent activations
        kk2 = big.tile([128, S], bf16, tag="kk2")
        v_sb = big.tile([128, NJ, 128], bf16, tag="v")
        nc.vector.memset(v_sb, 0.0)
        nc.vector.memset(v_sb[:, :, 64:65], 1.0)
        rkT_sb = big.tile([128, NJ], f32, tag="rkT")

        # xt prefetch: one si-block = 16 [128,512] chunks; keep 2 blocks in
        # flight (tag bufs=32).
        xts = {}

        def load_x(b):
            sp = slice(b * 512, (b + 1) * 512)
            ts = []
            for hc in range(HC):
                xt = xpool.tile([128, 512], bf16, tag="xt", bufs=32,
                                name=f"xt{b}_{hc}")
                nc.gpsimd.dma_start(out=xt, in_=xT_d[hc * 128:(hc + 1) * 128, sp])
                ts.append(xt)
            xts[b] = ts

        # ---------------- resident weights / tables
        # Everything on the gpsimd ring (spreads transfers over the 14-queue
        # pool).  Per-hc W tiles interleaved with xt(0) chunks so QKV(0)
        # matmul hc can start as soon as its own W/x chunks land; tables and
        # Wo are emitted after the critical-path loads (needed later).
        W_hc = [big.tile([128, 512], bf16, tag=f"W{hc}", name=f"W{hc}")
                for hc in range(HC)]

        def load_w_x0():
            sp = slice(0, 512)
            ts = []
            for hc in range(HC):
                nc.gpsimd.dma_start(out=W_hc[hc],
                                    in_=W_d[hc * 128:(hc + 1) * 128, :])
                xt = xpool.tile([128, 512], bf16, tag="xt", bufs=32,
                                name=f"xt0_{hc}")
                nc.gpsimd.dma_start(out=xt, in_=xT_d[hc * 128:(hc + 1) * 128, sp])
                ts.append(xt)
            xts[0] = ts

        def load_tables():
            def pair_table(src_d, tag):
                t = big.tile([128, S], bf16, tag=tag, name=tag)
                src = src_d.ap()
                ap2 = bass.AP(tensor=src.tensor, offset=src.offset,
                              ap=[[0, 2]] + list(src.ap))
                nc.gpsimd.dma_start(out=t, in_=ap2)
                return t

            cosq_sb = pair_table(cosq_d, "cosq")
            cosk_sb = big.tile([64, S], bf16, tag="cosk")
            nc.gpsimd.dma_start(out=cosk_sb, in_=cosk_d[:, :])
            sinq_sb = pair_table(sinq_d, "sinq")
            sink_sb = big.tile([64, S], bf16, tag="sink")
            nc.gpsimd.dma_start(out=sink_sb, in_=sink_d[:, :])
            return cosq_sb, sinq_sb, cosk_sb, sink_sb

        # per-block state handed across pipeline stages
        ps_cc_s, ps_g_s = {}, {}
        qr_s, kr_s, vr_s, sig_s = {}, {}, {}, {}

        def qkv(b):
            """QKV projection matmuls for si-block b (PE only)."""
            ps_cc = [psum.tile([128, 512], f32, tag="qkv3", bufs=3,
                               name=f"pscc{b}_{cc}") for cc in range(3)]
            ps_g = psum.tile([128, 512], f32, tag="gate", bufs=1,
                             name=f"psg{b}")
            for hc in range(HC):
                xt = xts[b][hc]
                st = (hc == 0)
                fin = (hc == HC - 1)
                for cc in range(3):
                    nc.tensor.matmul(ps_cc[cc][:],
                                     W_hc[hc][:, cc * 128:(cc + 1) * 128],
                                     xt, start=st, stop=fin)
                nc.tensor.matmul(ps_g[:], W_hc[hc][:, 384:512], xt,
                                 start=st, stop=fin)
            ps_cc_s[b] = ps_cc
            ps_g_s[b] = ps_g

        def extract(b):
            """Pull QKV(b) out of PSUM (ACT copies + gate exps)."""
            ps_cc, ps_g = ps_cc_s[b], ps_g_s[b]
            qr = [rawp.tile([128, 512], bf16, tag=f"qr{p}", name=f"qr{b}_{p}")
                  for p in range(2)]
            kr = rawp.tile([64, 512], bf16, tag="kr", name=f"kr{b}")
            vr = rawp.tile([64, 512], bf16, tag="vr", name=f"vr{b}")
            for p in range(2):
                nc.scalar.copy(qr[p], ps_cc[p][:])
            nc.scalar.copy(kr, ps_cc[2][0:64, :])
            nc.scalar.copy(vr, ps_cc[2][64:128, :])
            # exp(-gate) per head, packed at rows 0/32/64/96 (legal SBUF
            # partition starts) of one tile; PSUM row slices are exempt from
            # the partition-start rule so ps_g can be read per-head.
            sig_q = smal.tile([128, 512], f32, tag="sig", bufs=2,
                              name=f"eg{b}")
            for p in range(2):
                for hh in range(2):
                    r = 64 * p + 32 * hh
                    nc.scalar.activation(sig_q[r:r + 1, :],
                                         ps_g[r:r + 1, :],
                                         AF.Exp, scale=-1.0)
            qr_s[b], kr_s[b], vr_s[b], sig_s[b] = qr, kr, vr, sig_q

        rms_st = {}

        def rms_phase1(b):
            """RMS-scale matmuls + Newton seeds + k-side chain.  Emitted
            before QKV(b+1) so the Newton/RoPE chains (phase 2) can run on
            DVE/gpsimd while the PE crunches the next projection."""
            qr, kr = qr_s[b], kr_s[b]

            sqs = [None, None]
            pss = [None, None]
            y0s = [None, None]
            for p in range(2):
                eng = nc.vector if p == 0 else nc.gpsimd
                sq = sqp.tile([128, 512], bf16, tag=f"sq{p}", name=f"sq{b}_{p}")
                eng.tensor_mul(sq, qr[p], qr[p])
                sqs[p] = sq
            ksq = sqp.tile([64, 512], bf16, tag="ksq", name=f"ksq{b}")
            nc.vector.tensor_mul(ksq, kr, kr)
            for p in range(2):
                ps_rq = psum.tile([128, 512], f32, tag="sc", bufs=2,
                                  name=f"psrq{b}_{p}")
                nc.tensor.matmul(ps_rq[:], esel2, sqs[p], start=True, stop=True)
                pss[p] = ps_rq
            ps_rk = psum.tile([128, 4], f32, tag="sc", bufs=2, name=f"psrk{b}")
            for j in range(4):
                nc.tensor.matmul(ps_rk[:, j:j + 1],
                                 ksq[:, j * 128:(j + 1) * 128],
                                 ones[0:64, :], start=True, stop=True)
            a_t = [None, None]
            for p in range(2):
                y0 = rnp.tile([128, 512], f32, tag=f"rnB{p}", name=f"y0{b}_{p}")
                nc.scalar.activation(y0, pss[p][:].bitcast(u32), AF.Exp,
                                     bias=b_rsq, scale=-0.5 * EXPBIT_SCALE)
                y0s[p] = y0
                if p == 0:
                    a_t[p] = pss[p]
                else:
                    # gpsimd cannot read PSUM; stage the sums in SBUF
                    a_t[p] = rnp.tile([128, 512], f32, tag="rqs",
                                      name=f"rqs{b}")
                    nc.scalar.copy(a_t[p], pss[p][:])

            # k-side Newton ([128,4] ops are ~free) -> rkT ready early for
            # the attention exps
            yk = smal.tile([128, 4], f32, tag="smB", name=f"yk{b}")
            nc.scalar.activation(yk, ps_rk[:].bitcast(u32), AF.Exp,
                                 bias=b_rsq, scale=-0.5 * EXPBIT_SCALE)
            for it in range(2):
                last = (it == 1)
                tk = smal.tile([128, 4], f32, tag="smA", name=f"tk{b}")
                nc.vector.tensor_mul(tk, ps_rk[:], yk)
                nc.vector.tensor_mul(tk, tk, yk)
                nc.vector.tensor_scalar(tk, tk,
                                        (-0.5 * SCALE / HD) if last else (-0.5 / HD),
                                        (1.5 * SCALE) if last else 1.5,
                                        mybir.AluOpType.mult, mybir.AluOpType.add)
                if last:
                    nc.vector.tensor_mul(rkT_sb[:, b * 4:(b + 1) * 4], yk, tk)
                else:
                    ykn = smal.tile([128, 4], f32, tag="smB", name=f"ykn{b}")
                    nc.vector.tensor_mul(ykn, yk, tk)
                    yk = ykn
            rms_st[b] = (a_t, y0s)

        def rope_phase2(b):
            """Newton iterations + RoPE -> qf, kk2.  DVE handles the p=0
            chain and k; gpsimd handles the p=1 Newton and all rotate-half
            copies.  No PE work here — runs concurrently with QKV(b+1)."""
            sp = slice(b * 512, (b + 1) * 512)
            qr, kr = qr_s[b], kr_s[b]
            a_t, y0s = rms_st.pop(b)

            # rotate-half staging (cross-partition copies: DVE only — gpsimd
            # Q7 cores are hardwired to their own 16-partition slice and
            # cannot shuffle across partitions)
            qss = [None, None]
            for p in range(2):
                qs = tmpp.tile([128, 512], bf16, tag=f"qs{p}",
                               name=f"qs{b}_{p}")
                for g in range(2):
                    bb = g * 64
                    nc.vector.tensor_copy(qs[bb:bb + 32, :],
                                          qr[p][bb + 32:bb + 64, :])
                    nc.vector.tensor_copy(qs[bb + 32:bb + 64, :],
                                          qr[p][bb:bb + 32, :])
                qss[p] = qs
            ks = tmpp.tile([64, 512], bf16, tag="ks", name=f"ks{b}")
            nc.vector.tensor_copy(ks[0:32, :], kr[32:64, :])
            nc.vector.tensor_copy(ks[32:64, :], kr[0:32, :])

            rq = [None, None]
            for p in range(2):
                eng = nc.vector if p == 0 else nc.gpsimd
                y0 = y0s[p]
                if p == 0:
                    # z = a*y0 is the only (slow) PSUM read; the bank frees
                    # immediately and z*tn tracks a*y_n in SBUF thereafter
                    z = rnp.tile([128, 512], f32, tag="rnZ", name=f"z{b}")
                    eng.tensor_mul(z, a_t[p][:], y0)
                    src_a = z
                else:
                    src_a = a_t[p]
                for it in range(1):
                    tn = rnp.tile([128, 512], f32, tag=f"rnA{p}",
                                  name=f"tn{b}_{p}")
                    if p == 0:
                        eng.tensor_mul(tn, src_a[:], y0)
                    else:
                        eng.tensor_mul(tn, src_a[:], y0)
                        eng.tensor_mul(tn, tn, y0)
                    if p == 0:
                        pass
                    eng.tensor_scalar(tn, tn, -0.5 / HD, 1.5,
                                      mybir.AluOpType.mult,
                                      mybir.AluOpType.add)
                    yn = rnp.tile([128, 512], f32, tag=f"rnB{p}",
                                  name=f"yn{b}_{p}")
                    eng.tensor_mul(yn, y0, tn)
                    y0 = yn
                rq[p] = y0

            qf = [rawp.tile([128, 512], bf16, tag=f"qf{p}", name=f"qf{b}_{p}",
                            bufs=2) for p in range(2)]

            def rope_q(p):
                t1 = tmpp.tile([128, 512], bf16, tag="t1")
                nc.vector.tensor_mul(t1, qr[p], cosq_sb[:, sp])
                t2 = tmpp.tile([128, 512], bf16, tag="t2")
                nc.vector.tensor_mul(t2, qss[p], sinq_sb[:, sp])
                nc.vector.tensor_add(t2, t1, t2)
                nc.vector.tensor_mul(qf[p], t2, rq[p])

            # p=0 q first (needed by the first scores), then k (needed by
            # every scores), then p=1 q (needed only mid-attention)
            rope_q(0)
            t1k = tmpp.tile([64, 512], bf16, tag="t1")
            nc.vector.tensor_mul(t1k, kr, cosk_sb[:, sp])
            t2k = tmpp.tile([64, 512], bf16, tag="t2")
            nc.vector.tensor_mul(t2k, ks, sink_sb[:, sp])
            nc.vector.tensor_add(kk2[0:64, sp], t1k, t2k)
            nc.vector.tensor_copy(kk2[64:128, sp], kk2[0:64, sp])
            rope_q(1)
            return qf

        def vtransp(b):
            vr = vr_s[b]
            for j in range(4):
                J = b * 4 + j
                ps_v = psum.tile([128, 64], bf16, tag="sc", bufs=2,
                                 name=f"psv{b}_{j}")
                nc.tensor.transpose(ps_v[:], vr[:, j * 128:(j + 1) * 128], id64)
                nc.scalar.copy(v_sb[:, J, 0:64], ps_v[:])

        def attention(b, qf):
            B = b
            sig_q = sig_s[b]
            at = [rawp.tile([128, 512], bf16, tag=f"at{p}", name=f"at{b}_{p}",
                            bufs=2) for p in range(2)]
            attcp = [None, None]
            u_q = smal.tile([128, 512], f32, tag="u", bufs=2, name=f"u{b}")
            nc.vector.memset(u_q, 1.0)

            for p in range(2):
                ps_att = [psum.tile([128, 512], f32, tag="att", bufs=2,
                                    name=f"psatt{b}_{p}_{hh}") for hh in range(2)]
                for J in range(4 * B + 4):
                    off = max(0, (J - 4 * B) * 128)
                    ex = []
                    for hh in range(2):
                        rb = hh * 64
                        ps_s = psum.tile([128, 512], f32, tag="sc", bufs=2,
                                         name="pss")
                        nc.tensor.matmul(
                            ps_s[:, off:512],
                            kk2[rb:rb + 64, J * 128:(J + 1) * 128],
                            qf[p][rb:rb + 64, off:512],
                            start=True, stop=True,
                            tile_position=(rb, 0))
                        et = expp.tile([128, 512], bf16, tag="expT", bufs=5,
                                       name="et")
                        nc.scalar.activation(et[:, off:512], ps_s[:, off:512],
                                             AF.Exp, scale=rkT_sb[:, J:J + 1])
                        if off > 0 or J == 4 * B:
                            nc.vector.tensor_mul(et[:, off:off + 128],
                                                 et[:, off:off + 128], tri)
                        ex.append(et)
                    for hh in range(2):
                        nc.tensor.matmul(
                            ps_att[hh][:, off:512],
                            v_sb[:, J, :],
                            ex[hh][:, off:512],
                            start=(J == 0), stop=(J == 4 * B + 3))

                # drain ps_att immediately: PV values to SBUF (ACT), and the
                # scale chain u=(1+exp(-gate))*den -> s=1/u (DVE, direct PSUM
                # row reads + fast reciprocal).  For p=0 this hides entirely
                # under the p=1 J-loop.
                acp = bcp.tile([128, 512], f32, tag="attcp", name=f"acp{b}_{p}")
                for hh in range(2):
                    r = 64 * p + 32 * hh
                    nc.scalar.copy(acp[64 * hh:64 * hh + 64, :],
                                   ps_att[hh][0:64, :])
                    nc.vector.scalar_tensor_tensor(u_q[r:r + 1, :],
                                                   sig_q[r:r + 1, :], 1.0,
                                                   ps_att[hh][64:65, :],
                                                   mybir.AluOpType.add,
                                                   mybir.AluOpType.mult)
                attcp[p] = acp

            # packed Newton reciprocal: all four denominators in one chain
            s_y = smal.tile([128, 512], f32, tag="sy", bufs=2, name=f"sy{b}")
            nc.scalar.activation(s_y, u_q[:].bitcast(u32), AF.Exp,
                                 bias=b_rcp, scale=-EXPBIT_SCALE)
            for it in range(2):  # cheap [128,512] ops; keep 2 for accuracy
                tu = smal.tile([128, 512], f32, tag="tu", bufs=2,
                               name=f"tu{b}")
                nc.vector.tensor_mul(tu, u_q, s_y)
                nc.vector.tensor_scalar(tu, tu, -1.0, 2.0,
                                        mybir.AluOpType.mult,
                                        mybir.AluOpType.add)
                s_n = smal.tile([128, 512], f32, tag="sy", bufs=2,
                                name=f"sn{b}")
                nc.vector.tensor_mul(s_n, s_y, tu)
                s_y = s_n

            # broadcast scales to head rows on the PE (two accumulated K=1
            # matmuls) and apply in one [128,512] multiply per head pair
            for p in range(2):
                sbc_ps = psum.tile([128, 512], f32, tag="sc", bufs=2,
                                   name=f"sbc{b}_{p}")
                nc.tensor.matmul(sbc_ps[:], selp[p], s_y, start=True,
                                 stop=True)
                nc.vector.tensor_mul(at[p], attcp[p], sbc_ps[:])
            return at

        def outproj(b, at):
            B = b
            for ss in range(4 * B, 4 * B + 4):
                ls = (ss - 4 * B) * 128
                for qtr in range(4):
                    ps_o = psum.tile([128, 512], f32, tag="qkv3", bufs=3,
                                     name="pso")
                    nc.tensor.matmul(ps_o[:], at[0][:, ls:ls + 128],
                                     Wo_sb[:, 0, qtr * 512:(qtr + 1) * 512],
                                     start=True, stop=False)
                    nc.tensor.matmul(ps_o[:], at[1][:, ls:ls + 128],
                                     Wo_sb[:, 1, qtr * 512:(qtr + 1) * 512],
                                     start=False, stop=True)
                    ot = outs.tile([128, 512], f32, tag="ot")
                    if qtr % 2 == 0:
                        nc.scalar.copy(ot, ps_o[:])
                    else:
                        nc.vector.tensor_copy(ot, ps_o[:])
                    nc.gpsimd.dma_start(
                        out=out_d[ss * 128:(ss + 1) * 128,
                                  qtr * 512:(qtr + 1) * 512],
                        in_=ot)

        # ---------------- pipelined schedule
        load_w_x0()
        make_identity(nc, id64)
        make_upper_triangular(nc, tri, val=1.0, diag=True)
        qkv(0)
        extract(0)
        load_x(1)
        cosq_sb, sinq_sb, cosk_sb, sink_sb = load_tables()
        Wo_sb = big.tile([128, 2, H], bf16, tag="Wo")
        nc.gpsimd.dma_start(out=Wo_sb, in_=Wo_d.ap().rearrange(
            "(cc p) h -> p cc h", p=128))
        at_prev = None
        for B in range(NB):
            rms_phase1(B)
            if B + 1 < NB:
                qkv(B + 1)
            vtransp(B)
            qf = rope_phase2(B)
            if B + 2 < NB:
                load_x(B + 2)
            if at_prev is not None:
                outproj(B - 1, at_prev)
            at_prev = attention(B, qf)
            if B + 1 < NB:
                extract(B + 1)
        outproj(NB - 1, at_prev)

    nc.compile()
    return nc


def _get_nc():
    if "nc" not in _BUILT:
        _BUILT["nc"] = _build_nc()
    return _BUILT["nc"]


# ---------------------------------------------------------------- entry point
def _install_ntff_hook():
    import types
    try:
        import antenv
        if "antenv.axon_hooks" in sys.modules:
            return True
        mod = types.ModuleType("antenv.axon_hooks")
        holder = [None]
        mod.set_axon_ntff_profile_hook = lambda h: holder.__setitem__(0, h)
        mod.get_axon_ntff_profile_hook = lambda: holder[0]
        sys.modules["antenv.axon_hooks"] = mod
        antenv.axon_hooks = mod
        from trn_agent_boot.trn_boot import _ntff_profile_via_ctypes
        hook = _ntff_profile_via_ctypes("/opt/axon/libaxon_pjrt.so")
        if hook is None:
            return False
        mod.set_axon_ntff_profile_hook(hook)
        return True
    except Exception:
        return False


def kernel(hidden_states, Wq, Wk, Wv, Wo, g_q, g_k):
    global LAST_EXEC_NS
    from concourse.bass_utils import run_bass_kernel_spmd

    in_maps = _host_prep(hidden_states, Wq, Wk, Wv, Wo, g_q, g_k)
    nc = _get_nc()
    trace = os.environ.get("KERNEL_TRACE", "0") == "1"
    if trace:
        trace = _install_ntff_hook()
    res = run_bass_kernel_spmd(nc, in_maps, list(range(NCORES)), trace=trace)
    LAST_EXEC_NS = res.exec_time_ns
    out = np.zeros((S, H), np.float32)
    for c in range(NCORES):
        out += res.results[c]["out"]
    return out.reshape(1, S, H).astype(np.float32)


# revision 51
# speedup vs baseline: 1.0380x; 1.0246x over previous
"""GatedAttention TRN2 kernel — 8-core tensor-parallel (1 kv-head group per core).

Self-contained: host-side shard/layout prep + Bass/Tile kernel + gather.

Per-core dataflow (all device tensors feature-on-partition, "T" layouts):
  qkvT = W_c.T @ xT           (bf16 matmuls, PSUM accumulation, FWL-friendly
  128-col stationaries; per-head gate columns at partitions 0/32/64/96)
  q-RMS scale via block-diag selector matmul (the M=128 selector both sums
  squares per head and broadcasts the sum to all 64 head rows for free);
  Newton-rsqrt on [128,512] tiles split across DVE (p=0) and gpsimd (p=1)
  RoPE in bf16 on DVE (2x rate) with host-prefolded bf16 cos/sin tables
  scoresT[sj,si] per head, row-quadrant head pairs on the PE array
  exp on ACT with per-partition scale = 0.125 * rsqrt(mean k^2)
  P@V with V padded to M=128 (ones col 64 -> fused softmax denominators)
  u = (1+exp(-gate))*den read per-head straight from PSUM row 64 into a
  row-packed [128,512] tile; one packed Newton reciprocal chain; per-head
  broadcast back to 64 rows via a selector matmul on the PE
  out_partial = attnT_scaled.T @ Wo_c ; host sums the 8 partials.

Software pipeline: per block B the emission order is rms-matmuls(B),
v-transpose(B), QKV(B+1), rope(B) (DVE/gpsimd only, hidden under QKV(B+1)
on the PE), x-prefetch(B+2), outproj(B-1), attention(B), extract(B+1).
All bulk DMA (weights, tables, x tiles, output stores) runs on the gpsimd
SWDGE ring which spreads descriptors over the 14-queue pool; the 2-queue
sync ring carries only tiny constants.  Engine-SBUF access patterns must
start at partitions 0/32/64/96; cross-partition data movement is DVE-only
(gpsimd Q7 cores cannot shuffle across their 16-partition slices).
"""
import math
import os
import sys
import numpy as np
import ml_dtypes

BF16 = ml_dtypes.bfloat16

H, NH, KVH, HD = 2048, 32, 8, 64
G = NH // KVH          # 4 q heads per core
S = 2048
EPS = 1e-6
THETA = 1000000.0
SCALE = 1.0 / math.sqrt(HD)
NCORES = 8
HC = H // 128          # 16 h-chunks
NB = S // 512          # 4 si-blocks
NJ = S // 128          # 16 sj-chunks

_BUILT = {}
LAST_EXEC_NS = None


# ---------------------------------------------------------------- host prep
def _host_prep(hidden_states, Wq, Wk, Wv, Wo, g_q, g_k):
    x = np.ascontiguousarray(np.asarray(hidden_states, np.float32).reshape(S, H))
    Wq = np.asarray(Wq, np.float32)
    Wk = np.asarray(Wk, np.float32)
    Wv = np.asarray(Wv, np.float32)
    Wo = np.asarray(Wo, np.float32)
    g_q = np.asarray(g_q, np.float32)
    g_k = np.asarray(g_k, np.float32)

    xT = np.ascontiguousarray(x.T).astype(BF16)

    inv_freq = 1.0 / (THETA ** (np.arange(0, HD, 2, dtype=np.float32) / HD))
    pos = np.arange(S, dtype=np.float32)
    emb = np.concatenate([pos[:, None] * inv_freq[None, :]] * 2, axis=-1)  # [S,64]
    cos = np.cos(emb).T.astype(np.float32)   # [64, S]
    sin = np.sin(emb).T.astype(np.float32)
    sign = np.where(np.arange(HD) < HD // 2, -1.0, 1.0).astype(np.float32)[:, None]
    cosq = np.ascontiguousarray(cos * g_q[:, None]).astype(BF16)
    sinq = np.ascontiguousarray(sin * sign * np.roll(g_q, -32)[:, None]).astype(BF16)
    cosk = np.ascontiguousarray(cos * g_k[:, None]).astype(BF16)
    sink = np.ascontiguousarray(sin * sign * np.roll(g_k, -32)[:, None]).astype(BF16)

    in_maps = []
    for c in range(NCORES):
        Wq_g = Wq[:, c * (G * HD + G):(c + 1) * (G * HD + G)]
        gpad = np.zeros((H, 128), np.float32)
        for p in range(2):
            for hh in range(2):
                # gate for head (p,hh) lands on PSUM partition 64p+32hh — a
                # legal SBUF/PSUM access-start for the per-head exp reads
                gpad[:, 64 * p + 32 * hh] = Wq_g[:, G * HD + 2 * p + hh]
        W_c = np.ascontiguousarray(np.concatenate(
            [Wq_g[:, :G * HD],
             Wk[:, c * HD:(c + 1) * HD],
             Wv[:, c * HD:(c + 1) * HD],
             gpad], axis=1))                                   # [H, 512]
        Wo_c = np.ascontiguousarray(Wo[c * G * HD:(c + 1) * G * HD, :])  # [256,H]
        in_maps.append({"xT": xT, "W": W_c.astype(BF16), "Wo": Wo_c.astype(BF16),
                        "cosq": cosq, "sinq": sinq, "cosk": cosk, "sink": sink})
    return in_maps


# ---------------------------------------------------------------- bass build
def _build_nc():
    import concourse.bass as bass
    import concourse.mybir as mybir
    import concourse.tile as tile
    from concourse import bacc
    from concourse.masks import make_identity, make_upper_triangular

    dt = mybir.dt
    f32 = dt.float32
    bf16 = dt.bfloat16
    AF = mybir.ActivationFunctionType

    nc = bacc.Bacc("TRN2", target_bir_lowering=False, debug=False,
                   num_devices=NCORES)

    xT_d = nc.dram_tensor("xT", [H, S], bf16, kind="ExternalInput")
    W_d = nc.dram_tensor("W", [H, 512], bf16, kind="ExternalInput")
    Wo_d = nc.dram_tensor("Wo", [G * HD, H], bf16, kind="ExternalInput")
    cosq_d = nc.dram_tensor("cosq", [HD, S], bf16, kind="ExternalInput")
    sinq_d = nc.dram_tensor("sinq", [HD, S], bf16, kind="ExternalInput")
    cosk_d = nc.dram_tensor("cosk", [HD, S], bf16, kind="ExternalInput")
    sink_d = nc.dram_tensor("sink", [HD, S], bf16, kind="ExternalInput")
    out_d = nc.dram_tensor("out", [S, H], f32, kind="ExternalOutput")

    import contextlib
    with tile.TileContext(nc) as tc, contextlib.ExitStack() as ctx:
        const = ctx.enter_context(tc.tile_pool(name="const", bufs=1))
        big = ctx.enter_context(tc.tile_pool(name="big", bufs=1))
        xpool = ctx.enter_context(tc.tile_pool(name="xp", bufs=32))
        rawp = ctx.enter_context(tc.tile_pool(name="raw", bufs=2))
        tmpp = ctx.enter_context(tc.tile_pool(name="tmp", bufs=2))
        sqp = ctx.enter_context(tc.tile_pool(name="sq", bufs=2))
        rnp = ctx.enter_context(tc.tile_pool(name="rn", bufs=2))
        bcp = ctx.enter_context(tc.tile_pool(name="bc", bufs=2))
        expp = ctx.enter_context(tc.tile_pool(name="expp", bufs=4))
        outs = ctx.enter_context(tc.tile_pool(name="outs", bufs=3))
        smal = ctx.enter_context(tc.tile_pool(name="smal", bufs=2))
        psum = ctx.enter_context(tc.tile_pool(name="ps", bufs=1, space="PSUM"))

        # ---------------- constants (id64/tri are built on gpsimd, so they
        # are emitted after the critical startup loads — see schedule)
        id64 = const.tile([64, 64], bf16, tag="id64")
        tri = const.tile([128, 128], bf16, tag="tri")
        ones = const.tile([128, 1], bf16, tag="ones")
        nc.vector.memset(ones, 1.0)
        # block-diagonal selector: sums 64-row head blocks AND broadcasts the
        # result back to all 64 rows of the head (out partition p gets the sum
        # over contraction rows of the same head).
        esel2 = const.tile([128, 128], bf16, tag="esel2")
        nc.vector.memset(esel2, 0.0)
        nc.vector.memset(esel2[0:64, 0:64], 1.0)
        nc.vector.memset(esel2[64:128, 64:128], 1.0)
        # per-head scale row broadcast: sel_p[c,m]=1 where source row c feeds
        # head rows m (rows 0/32/64/96 are legal memset partition starts)
        selp = [const.tile([128, 128], f32, tag=f"sel{p}",
                name=f"sel{p}") for p in range(2)]
        for p in range(2):
            nc.vector.memset(selp[p], 0.0)
            nc.vector.memset(selp[p][64 * p:64 * p + 1, 0:64], 1.0)
            nc.vector.memset(selp[p][64 * p + 32:64 * p + 33, 64:128], 1.0)
        SIGMA = 0.0430
        EXPBIT_SCALE = math.log(2.0) / (1 << 23)
        b_rsq = const.tile([128, 1], f32, tag="brsq")
        nc.vector.memset(b_rsq, 0.5 * math.log(2.0) * (127 + SIGMA + 6))
        b_rcp = const.tile([128, 1], f32, tag="brcp")
        nc.vector.memset(b_rcp, math.log(2.0) * (127 + SIGMA))
        u32 = dt.uint32

        # ---------------- persistent activations
        kk2 = big.tile([128, S], bf16, tag="kk2")
        v_sb = big.tile([128, NJ, 128], bf16, tag="v")
        nc.vector.memset(v_sb, 0.0)
        nc.vector.memset(v_sb[:, :, 64:65], 1.0)
        rkT_sb = big.tile([128, NJ], f32, tag="rkT")

        # xt prefetch: one si-block = 16 [128,512] chunks; keep 2 blocks in
        # flight (tag bufs=32).
        xts = {}

        def load_x(b):
            sp = slice(b * 512, (b + 1) * 512)
            ts = []
            for hc in range(HC):
                xt = xpool.tile([128, 512], bf16, tag="xt", bufs=32,
                                name=f"xt{b}_{hc}")
                nc.gpsimd.dma_start(out=xt, in_=xT_d[hc * 128:(hc + 1) * 128, sp])
                ts.append(xt)
            xts[b] = ts

        # ---------------- resident weights / tables
        # Everything on the gpsimd ring (spreads transfers over the 14-queue
        # pool).  Per-hc W tiles interleaved with xt(0) chunks so QKV(0)
        # matmul hc can start as soon as its own W/x chunks land; tables and
        # Wo are emitted after the critical-path loads (needed later).
        W_hc = [big.tile([128, 512], bf16, tag=f"W{hc}", name=f"W{hc}")
                for hc in range(HC)]

        def load_w_x0():
            sp = slice(0, 512)
            ts = []
            for hc in range(HC):
                nc.gpsimd.dma_start(out=W_hc[hc],
                                    in_=W_d[hc * 128:(hc + 1) * 128, :])
                xt = xpool.tile([128, 512], bf16, tag="xt", bufs=32,
                                name=f"xt0_{hc}")
                nc.gpsimd.dma_start(out=xt, in_=xT_d[hc * 128:(hc + 1) * 128, sp])
                ts.append(xt)
            xts[0] = ts

        def load_tables():
            def pair_table(src_d, tag):
                t = big.tile([128, S], bf16, tag=tag, name=tag)
                src = src_d.ap()
                ap2 = bass.AP(tensor=src.tensor, offset=src.offset,
                              ap=[[0, 2]] + list(src.ap))
                nc.gpsimd.dma_start(out=t, in_=ap2)
                return t

            cosq_sb = pair_table(cosq_d, "cosq")
            cosk_sb = big.tile([64, S], bf16, tag="cosk")
            nc.gpsimd.dma_start(out=cosk_sb, in_=cosk_d[:, :])
            sinq_sb = pair_table(sinq_d, "sinq")
            sink_sb = big.tile([64, S], bf16, tag="sink")
            nc.gpsimd.dma_start(out=sink_sb, in_=sink_d[:, :])
            return cosq_sb, sinq_sb, cosk_sb, sink_sb

        # per-block state handed across pipeline stages
        ps_cc_s, ps_g_s = {}, {}
        qr_s, kr_s, vr_s, sig_s = {}, {}, {}, {}

        def qkv(b):
            """QKV projection matmuls for si-block b (PE only)."""
            ps_cc = [psum.tile([128, 512], f32, tag="qkv3", bufs=3,
                               name=f"pscc{b}_{cc}") for cc in range(3)]
            ps_g = psum.tile([128, 512], f32, tag="gate", bufs=1,
                             name=f"psg{b}")
            for hc in range(HC):
                xt = xts[b][hc]
                st = (hc == 0)
                fin = (hc == HC - 1)
                for cc in range(3):
                    nc.tensor.matmul(ps_cc[cc][:],
                                     W_hc[hc][:, cc * 128:(cc + 1) * 128],
                                     xt, start=st, stop=fin)
                nc.tensor.matmul(ps_g[:], W_hc[hc][:, 384:512], xt,
                                 start=st, stop=fin)
            ps_cc_s[b] = ps_cc
            ps_g_s[b] = ps_g

        def extract(b):
            """Pull QKV(b) out of PSUM (ACT copies + gate exps)."""
            ps_cc, ps_g = ps_cc_s[b], ps_g_s[b]
            qr = [rawp.tile([128, 512], bf16, tag=f"qr{p}", name=f"qr{b}_{p}")
                  for p in range(2)]
            kr = rawp.tile([64, 512], bf16, tag="kr", name=f"kr{b}")
            vr = rawp.tile([64, 512], bf16, tag="vr", name=f"vr{b}")
            for p in range(2):
                nc.scalar.copy(qr[p], ps_cc[p][:])
            nc.scalar.copy(kr, ps_cc[2][0:64, :])
            nc.scalar.copy(vr, ps_cc[2][64:128, :])
            # exp(-gate) per head, packed at rows 0/32/64/96 (legal SBUF
            # partition starts) of one tile; PSUM row slices are exempt from
            # the partition-start rule so ps_g can be read per-head.
            sig_q = smal.tile([128, 512], f32, tag="sig", bufs=2,
                              name=f"eg{b}")
            for p in range(2):
                for hh in range(2):
                    r = 64 * p + 32 * hh
                    nc.scalar.activation(sig_q[r:r + 1, :],
                                         ps_g[r:r + 1, :],
                                         AF.Exp, scale=-1.0)
            qr_s[b], kr_s[b], vr_s[b], sig_s[b] = qr, kr, vr, sig_q

        rms_st = {}

        def rms_phase1(b):
            """RMS-scale matmuls + Newton seeds + k-side chain.  Emitted
            before QKV(b+1) so the Newton/RoPE chains (phase 2) can run on
            DVE/gpsimd while the PE crunches the next projection."""
            qr, kr = qr_s[b], kr_s[b]

            sqs = [None, None]
            pss = [None, None]
            y0s = [None, None]
            for p in range(2):
                sq = sqp.tile([128, 512], bf16, tag=f"sq{p}", name=f"sq{b}_{p}")
                nc.vector.tensor_mul(sq, qr[p], qr[p])
                sqs[p] = sq
            ksq = sqp.tile([64, 512], bf16, tag="ksq", name=f"ksq{b}")
            nc.vector.tensor_mul(ksq, kr, kr)
            for p in range(2):
                ps_rq = psum.tile([128, 512], f32, tag="sc", bufs=2,
                                  name=f"psrq{b}_{p}")
                nc.tensor.matmul(ps_rq[:], esel2, sqs[p], start=True, stop=True)
                pss[p] = ps_rq
            ps_rk = psum.tile([128, 4], f32, tag="sc", bufs=2, name=f"psrk{b}")
            for j in range(4):
                nc.tensor.matmul(ps_rk[:, j:j + 1],
                                 ksq[:, j * 128:(j + 1) * 128],
                                 ones[0:64, :], start=True, stop=True)
            a_t = [None, None]
            for p in range(2):
                y0 = rnp.tile([128, 512], f32, tag=f"rnB{p}", name=f"y0{b}_{p}")
                nc.scalar.activation(y0, pss[p][:].bitcast(u32), AF.Exp,
                                     bias=b_rsq, scale=-0.5 * EXPBIT_SCALE)
                y0s[p] = y0
                if p == 0:
                    a_t[p] = pss[p]
                else:
                    # gpsimd cannot read PSUM; stage the sums in SBUF
                    a_t[p] = rnp.tile([128, 512], f32, tag="rqs",
                                      name=f"rqs{b}")
                    nc.scalar.copy(a_t[p], pss[p][:])

            # k-side Newton ([128,4] ops are ~free) -> rkT ready early for
            # the attention exps
            yk = smal.tile([128, 4], f32, tag="smB", name=f"yk{b}")
            nc.scalar.activation(yk, ps_rk[:].bitcast(u32), AF.Exp,
                                 bias=b_rsq, scale=-0.5 * EXPBIT_SCALE)
            for it in range(2):
                last = (it == 1)
                tk = smal.tile([128, 4], f32, tag="smA", name=f"tk{b}")
                nc.vector.tensor_mul(tk, ps_rk[:], yk)
                nc.vector.tensor_mul(tk, tk, yk)
                nc.vector.tensor_scalar(tk, tk,
                                        (-0.5 * SCALE / HD) if last else (-0.5 / HD),
                                        (1.5 * SCALE) if last else 1.5,
                                        mybir.AluOpType.mult, mybir.AluOpType.add)
                if last:
                    nc.vector.tensor_mul(rkT_sb[:, b * 4:(b + 1) * 4], yk, tk)
                else:
                    ykn = smal.tile([128, 4], f32, tag="smB", name=f"ykn{b}")
                    nc.vector.tensor_mul(ykn, yk, tk)
                    yk = ykn
            rms_st[b] = (a_t, y0s)

        def rope_phase2(b):
            """Newton iterations + RoPE -> qf, kk2.  DVE handles the p=0
            chain and k; gpsimd handles the p=1 Newton and all rotate-half
            copies.  No PE work here — runs concurrently with QKV(b+1)."""
            sp = slice(b * 512, (b + 1) * 512)
            qr, kr = qr_s[b], kr_s[b]
            a_t, y0s = rms_st.pop(b)

            # rotate-half staging (cross-partition copies: DVE only — gpsimd
            # Q7 cores are hardwired to their own 16-partition slice and
            # cannot shuffle across partitions)
            qss = [None, None]
            for p in range(2):
                qs = tmpp.tile([128, 512], bf16, tag=f"qs{p}",
                               name=f"qs{b}_{p}")
                for g in range(2):
                    bb = g * 64
                    nc.vector.tensor_copy(qs[bb:bb + 32, :],
                                          qr[p][bb + 32:bb + 64, :])
                    nc.vector.tensor_copy(qs[bb + 32:bb + 64, :],
                                          qr[p][bb:bb + 32, :])
                qss[p] = qs
            ks = tmpp.tile([64, 512], bf16, tag="ks", name=f"ks{b}")
            nc.vector.tensor_copy(ks[0:32, :], kr[32:64, :])
            nc.vector.tensor_copy(ks[32:64, :], kr[0:32, :])

            rq = [None, None]
            for p in range(2):
                eng = nc.vector if p == 0 else nc.gpsimd
                y0 = y0s[p]
                if p == 0:
                    # z = a*y0 is the only (slow) PSUM read; the bank frees
                    # immediately and z*tn tracks a*y_n in SBUF thereafter
                    z = rnp.tile([128, 512], f32, tag="rnZ", name=f"z{b}")
                    eng.tensor_mul(z, a_t[p][:], y0)
                    src_a = z
                else:
                    src_a = a_t[p]
                for it in range(1):
                    tn = rnp.tile([128, 512], f32, tag=f"rnA{p}",
                                  name=f"tn{b}_{p}")
                    if p == 0:
                        eng.tensor_mul(tn, src_a[:], y0)
                    else:
                        eng.tensor_mul(tn, src_a[:], y0)
                        eng.tensor_mul(tn, tn, y0)
                    if p == 0:
                        pass
                    eng.tensor_scalar(tn, tn, -0.5 / HD, 1.5,
                                      mybir.AluOpType.mult,
                                      mybir.AluOpType.add)
                    yn = rnp.tile([128, 512], f32, tag=f"rnB{p}",
                                  name=f"yn{b}_{p}")
                    eng.tensor_mul(yn, y0, tn)
                    y0 = yn
                rq[p] = y0

            qf = [rawp.tile([128, 512], bf16, tag=f"qf{p}", name=f"qf{b}_{p}",
                            bufs=2) for p in range(2)]

            def rope_q(p):
                t1 = tmpp.tile([128, 512], bf16, tag="t1")
                nc.vector.tensor_mul(t1, qr[p], cosq_sb[:, sp])
                t2 = tmpp.tile([128, 512], bf16, tag="t2")
                nc.vector.tensor_mul(t2, qss[p], sinq_sb[:, sp])
                nc.vector.tensor_add(t2, t1, t2)
                nc.vector.tensor_mul(qf[p], t2, rq[p])

            # p=0 q first (needed by the first scores), then k (needed by
            # every scores), then p=1 q (needed only mid-attention)
            rope_q(0)
            t1k = tmpp.tile([64, 512], bf16, tag="t1")
            nc.vector.tensor_mul(t1k, kr, cosk_sb[:, sp])
            t2k = tmpp.tile([64, 512], bf16, tag="t2")
            nc.vector.tensor_mul(t2k, ks, sink_sb[:, sp])
            nc.vector.tensor_add(kk2[0:64, sp], t1k, t2k)
            nc.vector.tensor_copy(kk2[64:128, sp], kk2[0:64, sp])
            rope_q(1)
            return qf

        def vtransp(b):
            vr = vr_s[b]
            for j in range(4):
                J = b * 4 + j
                ps_v = psum.tile([128, 64], bf16, tag="sc", bufs=2,
                                 name=f"psv{b}_{j}")
                nc.tensor.transpose(ps_v[:], vr[:, j * 128:(j + 1) * 128], id64)
                nc.scalar.copy(v_sb[:, J, 0:64], ps_v[:])

        def attention(b, qf):
            B = b
            sig_q = sig_s[b]
            at = [rawp.tile([128, 512], bf16, tag=f"at{p}", name=f"at{b}_{p}",
                            bufs=2) for p in range(2)]
            attcp = [None, None]
            u_q = smal.tile([128, 512], f32, tag="u", bufs=2, name=f"u{b}")
            nc.vector.memset(u_q, 1.0)

            for p in range(2):
                ps_att = [psum.tile([128, 512], f32, tag="att", bufs=2,
                                    name=f"psatt{b}_{p}_{hh}") for hh in range(2)]
                for J in range(4 * B + 4):
                    off = max(0, (J - 4 * B) * 128)
                    ex = []
                    for hh in range(2):
                        rb = hh * 64
                        ps_s = psum.tile([128, 512], f32, tag="sc", bufs=2,
                                         name="pss")
                        nc.tensor.matmul(
                            ps_s[:, off:512],
                            kk2[rb:rb + 64, J * 128:(J + 1) * 128],
                            qf[p][rb:rb + 64, off:512],
                            start=True, stop=True,
                            tile_position=(rb, 0))
                        et = expp.tile([128, 512], bf16, tag="expT", bufs=5,
                                       name="et")
                        nc.scalar.activation(et[:, off:512], ps_s[:, off:512],
                                             AF.Exp, scale=rkT_sb[:, J:J + 1])
                        if off > 0 or J == 4 * B:
                            nc.vector.tensor_mul(et[:, off:off + 128],
                                                 et[:, off:off + 128], tri)
                        ex.append(et)
                    for hh in range(2):
                        nc.tensor.matmul(
                            ps_att[hh][:, off:512],
                            v_sb[:, J, :],
                            ex[hh][:, off:512],
                            start=(J == 0), stop=(J == 4 * B + 3))

                # drain ps_att immediately: PV values to SBUF (ACT), and the
                # scale chain u=(1+exp(-gate))*den -> s=1/u (DVE, direct PSUM
                # row reads + fast reciprocal).  For p=0 this hides entirely
                # under the p=1 J-loop.
                acp = bcp.tile([128, 512], f32, tag="attcp", name=f"acp{b}_{p}")
                for hh in range(2):
                    r = 64 * p + 32 * hh
                    nc.scalar.copy(acp[64 * hh:64 * hh + 64, :],
                                   ps_att[hh][0:64, :])
                    nc.vector.scalar_tensor_tensor(u_q[r:r + 1, :],
                                                   sig_q[r:r + 1, :], 1.0,
                                                   ps_att[hh][64:65, :],
                                                   mybir.AluOpType.add,
                                                   mybir.AluOpType.mult)
                attcp[p] = acp

            # packed Newton reciprocal: all four denominators in one chain
            s_y = smal.tile([128, 512], f32, tag="sy", bufs=2, name=f"sy{b}")
            nc.scalar.activation(s_y, u_q[:].bitcast(u32), AF.Exp,
                                 bias=b_rcp, scale=-EXPBIT_SCALE)
            for it in range(2):  # cheap [128,512] ops; keep 2 for accuracy
                tu = smal.tile([128, 512], f32, tag="tu", bufs=2,
                               name=f"tu{b}")
                nc.vector.tensor_mul(tu, u_q, s_y)
                nc.vector.tensor_scalar(tu, tu, -1.0, 2.0,
                                        mybir.AluOpType.mult,
                                        mybir.AluOpType.add)
                s_n = smal.tile([128, 512], f32, tag="sy", bufs=2,
                                name=f"sn{b}")
                nc.vector.tensor_mul(s_n, s_y, tu)
                s_y = s_n

            # broadcast scales to head rows on the PE (two accumulated K=1
            # matmuls) and apply in one [128,512] multiply per head pair
            for p in range(2):
                sbc_ps = psum.tile([128, 512], f32, tag="sc", bufs=2,
                                   name=f"sbc{b}_{p}")
                nc.tensor.matmul(sbc_ps[:], selp[p], s_y, start=True,
                                 stop=True)
                nc.vector.tensor_mul(at[p], attcp[p], sbc_ps[:])
            return at

        def outproj(b, at):
            B = b
            for ss in range(4 * B, 4 * B + 4):
                ls = (ss - 4 * B) * 128
                for qtr in range(4):
                    ps_o = psum.tile([128, 512], f32, tag="qkv3", bufs=3,
                                     name="pso")
                    nc.tensor.matmul(ps_o[:], at[0][:, ls:ls + 128],
                                     Wo_sb[:, 0, qtr * 512:(qtr + 1) * 512],
                                     start=True, stop=False)
                    nc.tensor.matmul(ps_o[:], at[1][:, ls:ls + 128],
                                     Wo_sb[:, 1, qtr * 512:(qtr + 1) * 512],
                                     start=False, stop=True)
                    ot = outs.tile([128, 512], f32, tag="ot")
                    if qtr % 2 == 0:
                        nc.scalar.copy(ot, ps_o[:])
                    else:
                        nc.vector.tensor_copy(ot, ps_o[:])
                    nc.gpsimd.dma_start(
                        out=out_d[ss * 128:(ss + 1) * 128,
                                  qtr * 512:(qtr + 1) * 512],
                        in_=ot)

        # ---------------- pipelined schedule
        load_w_x0()
        make_identity(nc, id64)
        make_upper_triangular(nc, tri, val=1.0, diag=True)
        qkv(0)
        extract(0)
        load_x(1)
        cosq_sb, sinq_sb, cosk_sb, sink_sb = load_tables()
        Wo_sb = big.tile([128, 2, H], bf16, tag="Wo")
        nc.gpsimd.dma_start(out=Wo_sb, in_=Wo_d.ap().rearrange(
            "(cc p) h -> p cc h", p=128))
        at_prev = None
        for B in range(NB):
            if B + 2 < NB:
                load_x(B + 2)
            rms_phase1(B)
            if B + 1 < NB:
                qkv(B + 1)
            vtransp(B)
            qf = rope_phase2(B)
            if at_prev is not None:
                outproj(B - 1, at_prev)
            at_prev = attention(B, qf)
            if B + 1 < NB:
                extract(B + 1)
        outproj(NB - 1, at_prev)

    nc.compile()
    return nc


def _get_nc():
    if "nc" not in _BUILT:
        _BUILT["nc"] = _build_nc()
    return _BUILT["nc"]


# ---------------------------------------------------------------- entry point
def _install_ntff_hook():
    import types
    try:
        import antenv
        if "antenv.axon_hooks" in sys.modules:
            return True
        mod = types.ModuleType("antenv.axon_hooks")
        holder = [None]
        mod.set_axon_ntff_profile_hook = lambda h: holder.__setitem__(0, h)
        mod.get_axon_ntff_profile_hook = lambda: holder[0]
        sys.modules["antenv.axon_hooks"] = mod
        antenv.axon_hooks = mod
        from trn_agent_boot.trn_boot import _ntff_profile_via_ctypes
        hook = _ntff_profile_via_ctypes("/opt/axon/libaxon_pjrt.so")
        if hook is None:
            return False
        mod.set_axon_ntff_profile_hook(hook)
        return True
    except Exception:
        return False


def kernel(hidden_states, Wq, Wk, Wv, Wo, g_q, g_k):
    global LAST_EXEC_NS
    from concourse.bass_utils import run_bass_kernel_spmd

    in_maps = _host_prep(hidden_states, Wq, Wk, Wv, Wo, g_q, g_k)
    nc = _get_nc()
    trace = os.environ.get("KERNEL_TRACE", "0") == "1"
    if trace:
        trace = _install_ntff_hook()
    res = run_bass_kernel_spmd(nc, in_maps, list(range(NCORES)), trace=trace)
    LAST_EXEC_NS = res.exec_time_ns
    out = np.zeros((S, H), np.float32)
    for c in range(NCORES):
        out += res.results[c]["out"]
    return out.reshape(1, S, H).astype(np.float32)


# revision 52
# speedup vs baseline: 1.0781x; 1.0386x over previous
"""GatedAttention TRN2 kernel — 8-core tensor-parallel (1 kv-head group per core).

Self-contained: host-side shard/layout prep + Bass/Tile kernel + gather.

Per-core dataflow (all device tensors feature-on-partition, "T" layouts):
  qkvT = W_c.T @ xT           (bf16 matmuls, PSUM accumulation, FWL-friendly
  128-col stationaries; per-head gate columns at partitions 0/32/64/96)
  q-RMS scale via block-diag selector matmul (the M=128 selector both sums
  squares per head and broadcasts the sum to all 64 head rows for free);
  Newton-rsqrt on [128,512] tiles split across DVE (p=0) and gpsimd (p=1)
  RoPE in bf16 on DVE (2x rate) with host-prefolded bf16 cos/sin tables
  scoresT[sj,si] per head, row-quadrant head pairs on the PE array
  exp on ACT with per-partition scale = 0.125 * rsqrt(mean k^2)
  P@V with V padded to M=128 (ones col 64 -> fused softmax denominators)
  u = (1+exp(-gate))*den read per-head straight from PSUM row 64 into a
  row-packed [128,512] tile; one packed Newton reciprocal chain; per-head
  broadcast back to 64 rows via a selector matmul on the PE
  out_partial = attnT_scaled.T @ Wo_c ; host sums the 8 partials.

Software pipeline: per block B the emission order is rms-matmuls(B),
v-transpose(B), QKV(B+1), rope(B) (DVE/gpsimd only, hidden under QKV(B+1)
on the PE), x-prefetch(B+2), outproj(B-1), attention(B), extract(B+1).
All bulk DMA (weights, tables, x tiles, output stores) runs on the gpsimd
SWDGE ring which spreads descriptors over the 14-queue pool; the 2-queue
sync ring carries only tiny constants.  Engine-SBUF access patterns must
start at partitions 0/32/64/96; cross-partition data movement is DVE-only
(gpsimd Q7 cores cannot shuffle across their 16-partition slices).
"""
import math
import os
import sys
import numpy as np
import ml_dtypes

BF16 = ml_dtypes.bfloat16

H, NH, KVH, HD = 2048, 32, 8, 64
G = NH // KVH          # 4 q heads per core
S = 2048
EPS = 1e-6
THETA = 1000000.0
SCALE = 1.0 / math.sqrt(HD)
NCORES = 8
HC = H // 128          # 16 h-chunks
NB = S // 512          # 4 si-blocks
NJ = S // 128          # 16 sj-chunks

_BUILT = {}
LAST_EXEC_NS = None


# ---------------------------------------------------------------- host prep
def _host_prep(hidden_states, Wq, Wk, Wv, Wo, g_q, g_k):
    x = np.ascontiguousarray(np.asarray(hidden_states, np.float32).reshape(S, H))
    Wq = np.asarray(Wq, np.float32)
    Wk = np.asarray(Wk, np.float32)
    Wv = np.asarray(Wv, np.float32)
    Wo = np.asarray(Wo, np.float32)
    g_q = np.asarray(g_q, np.float32)
    g_k = np.asarray(g_k, np.float32)

    xT = np.ascontiguousarray(x.T).astype(BF16)

    inv_freq = 1.0 / (THETA ** (np.arange(0, HD, 2, dtype=np.float32) / HD))
    pos = np.arange(S, dtype=np.float32)
    emb = np.concatenate([pos[:, None] * inv_freq[None, :]] * 2, axis=-1)  # [S,64]
    cos = np.cos(emb).T.astype(np.float32)   # [64, S]
    sin = np.sin(emb).T.astype(np.float32)
    sign = np.where(np.arange(HD) < HD // 2, -1.0, 1.0).astype(np.float32)[:, None]
    cosq = np.ascontiguousarray(cos * g_q[:, None]).astype(BF16)
    sinq = np.ascontiguousarray(sin * sign * np.roll(g_q, -32)[:, None]).astype(BF16)
    cosk = np.ascontiguousarray(cos * g_k[:, None]).astype(BF16)
    sink = np.ascontiguousarray(sin * sign * np.roll(g_k, -32)[:, None]).astype(BF16)

    in_maps = []
    for c in range(NCORES):
        Wq_g = Wq[:, c * (G * HD + G):(c + 1) * (G * HD + G)]
        gpad = np.zeros((H, 128), np.float32)
        for p in range(2):
            for hh in range(2):
                # gate for head (p,hh) lands on PSUM partition 64p+32hh — a
                # legal SBUF/PSUM access-start for the per-head exp reads
                gpad[:, 64 * p + 32 * hh] = Wq_g[:, G * HD + 2 * p + hh]
        W_c = np.ascontiguousarray(np.concatenate(
            [Wq_g[:, :G * HD],
             Wk[:, c * HD:(c + 1) * HD],
             Wv[:, c * HD:(c + 1) * HD],
             gpad], axis=1))                                   # [H, 512]
        Wo_c = np.ascontiguousarray(Wo[c * G * HD:(c + 1) * G * HD, :])  # [256,H]
        in_maps.append({"xT": xT, "W": W_c.astype(BF16), "Wo": Wo_c.astype(BF16),
                        "cosq": cosq, "sinq": sinq, "cosk": cosk, "sink": sink})
    return in_maps


# ---------------------------------------------------------------- bass build
def _build_nc():
    import concourse.bass as bass
    import concourse.mybir as mybir
    import concourse.tile as tile
    from concourse import bacc
    from concourse.masks import make_identity, make_upper_triangular

    dt = mybir.dt
    f32 = dt.float32
    bf16 = dt.bfloat16
    AF = mybir.ActivationFunctionType

    nc = bacc.Bacc("TRN2", target_bir_lowering=False, debug=False,
                   num_devices=NCORES)

    xT_d = nc.dram_tensor("xT", [H, S], bf16, kind="ExternalInput")
    W_d = nc.dram_tensor("W", [H, 512], bf16, kind="ExternalInput")
    Wo_d = nc.dram_tensor("Wo", [G * HD, H], bf16, kind="ExternalInput")
    cosq_d = nc.dram_tensor("cosq", [HD, S], bf16, kind="ExternalInput")
    sinq_d = nc.dram_tensor("sinq", [HD, S], bf16, kind="ExternalInput")
    cosk_d = nc.dram_tensor("cosk", [HD, S], bf16, kind="ExternalInput")
    sink_d = nc.dram_tensor("sink", [HD, S], bf16, kind="ExternalInput")
    out_d = nc.dram_tensor("out", [S, H], f32, kind="ExternalOutput")

    import contextlib
    with tile.TileContext(nc) as tc, contextlib.ExitStack() as ctx:
        const = ctx.enter_context(tc.tile_pool(name="const", bufs=1))
        big = ctx.enter_context(tc.tile_pool(name="big", bufs=1))
        xpool = ctx.enter_context(tc.tile_pool(name="xp", bufs=32))
        rawp = ctx.enter_context(tc.tile_pool(name="raw", bufs=2))
        tmpp = ctx.enter_context(tc.tile_pool(name="tmp", bufs=2))
        sqp = ctx.enter_context(tc.tile_pool(name="sq", bufs=2))
        rnp = ctx.enter_context(tc.tile_pool(name="rn", bufs=2))
        bcp = ctx.enter_context(tc.tile_pool(name="bc", bufs=2))
        expp = ctx.enter_context(tc.tile_pool(name="expp", bufs=4))
        outs = ctx.enter_context(tc.tile_pool(name="outs", bufs=3))
        smal = ctx.enter_context(tc.tile_pool(name="smal", bufs=2))
        psum = ctx.enter_context(tc.tile_pool(name="ps", bufs=1, space="PSUM"))

        # ---------------- constants (id64/tri are built on gpsimd, so they
        # are emitted after the critical startup loads — see schedule)
        id64 = const.tile([64, 64], bf16, tag="id64")
        tri = const.tile([128, 128], bf16, tag="tri")
        ones = const.tile([128, 1], bf16, tag="ones")
        nc.vector.memset(ones, 1.0)
        # block-diagonal selector: sums 64-row head blocks AND broadcasts the
        # result back to all 64 rows of the head (out partition p gets the sum
        # over contraction rows of the same head).
        esel2 = const.tile([128, 128], bf16, tag="esel2")
        nc.vector.memset(esel2, 0.0)
        nc.vector.memset(esel2[0:64, 0:64], 1.0)
        nc.vector.memset(esel2[64:128, 64:128], 1.0)
        # per-head scale row broadcast: sel_p[c,m]=1 where source row c feeds
        # head rows m (rows 0/32/64/96 are legal memset partition starts)
        selp = [const.tile([128, 128], f32, tag=f"sel{p}",
                name=f"sel{p}") for p in range(2)]
        for p in range(2):
            nc.vector.memset(selp[p], 0.0)
            nc.vector.memset(selp[p][64 * p:64 * p + 1, 0:64], 1.0)
            nc.vector.memset(selp[p][64 * p + 32:64 * p + 33, 64:128], 1.0)
        SIGMA = 0.0430
        EXPBIT_SCALE = math.log(2.0) / (1 << 23)
        b_rsq = const.tile([128, 1], f32, tag="brsq")
        nc.vector.memset(b_rsq, 0.5 * math.log(2.0) * (127 + SIGMA + 6))
        b_rcp = const.tile([128, 1], f32, tag="brcp")
        nc.vector.memset(b_rcp, math.log(2.0) * (127 + SIGMA))
        u32 = dt.uint32

        # ---------------- persistent activations
        kk2 = big.tile([128, S], bf16, tag="kk2")
        v_sb = big.tile([128, NJ, 128], bf16, tag="v")
        nc.vector.memset(v_sb, 0.0)
        nc.vector.memset(v_sb[:, :, 64:65], 1.0)
        rkT_sb = big.tile([128, NJ], f32, tag="rkT")

        # xt prefetch: one si-block = 16 [128,512] chunks; keep 2 blocks in
        # flight (tag bufs=32).
        xts = {}

        def load_x(b):
            sp = slice(b * 512, (b + 1) * 512)
            ts = []
            for hc in range(HC):
                xt = xpool.tile([128, 512], bf16, tag="xt", bufs=32,
                                name=f"xt{b}_{hc}")
                nc.gpsimd.dma_start(out=xt, in_=xT_d[hc * 128:(hc + 1) * 128, sp])
                ts.append(xt)
            xts[b] = ts

        # ---------------- resident weights / tables
        # Everything on the gpsimd ring (spreads transfers over the 14-queue
        # pool).  Per-hc W tiles interleaved with xt(0) chunks so QKV(0)
        # matmul hc can start as soon as its own W/x chunks land; tables and
        # Wo are emitted after the critical-path loads (needed later).
        W_hc = [big.tile([128, 512], bf16, tag=f"W{hc}", name=f"W{hc}")
                for hc in range(HC)]

        def load_w_x0():
            sp = slice(0, 512)
            ts = []
            for hc in range(HC):
                nc.gpsimd.dma_start(out=W_hc[hc],
                                    in_=W_d[hc * 128:(hc + 1) * 128, :])
                xt = xpool.tile([128, 512], bf16, tag="xt", bufs=32,
                                name=f"xt0_{hc}")
                nc.gpsimd.dma_start(out=xt, in_=xT_d[hc * 128:(hc + 1) * 128, sp])
                ts.append(xt)
            xts[0] = ts

        def load_tables():
            def pair_table(src_d, tag):
                t = big.tile([128, S], bf16, tag=tag, name=tag)
                src = src_d.ap()
                ap2 = bass.AP(tensor=src.tensor, offset=src.offset,
                              ap=[[0, 2]] + list(src.ap))
                nc.gpsimd.dma_start(out=t, in_=ap2)
                return t

            cosq_sb = pair_table(cosq_d, "cosq")
            cosk_sb = big.tile([64, S], bf16, tag="cosk")
            nc.gpsimd.dma_start(out=cosk_sb, in_=cosk_d[:, :])
            sinq_sb = pair_table(sinq_d, "sinq")
            sink_sb = big.tile([64, S], bf16, tag="sink")
            nc.gpsimd.dma_start(out=sink_sb, in_=sink_d[:, :])
            return cosq_sb, sinq_sb, cosk_sb, sink_sb

        # per-block state handed across pipeline stages
        ps_cc_s, ps_g_s = {}, {}
        qr_s, kr_s, vr_s, sig_s = {}, {}, {}, {}

        def qkv(b):
            """QKV projection matmuls for si-block b (PE only)."""
            ps_cc = [psum.tile([128, 512], f32, tag="qkv3", bufs=3,
                               name=f"pscc{b}_{cc}") for cc in range(3)]
            ps_g = psum.tile([128, 512], f32, tag="gate", bufs=1,
                             name=f"psg{b}")
            for hc in range(HC):
                xt = xts[b][hc]
                st = (hc == 0)
                fin = (hc == HC - 1)
                for cc in range(3):
                    nc.tensor.matmul(ps_cc[cc][:],
                                     W_hc[hc][:, cc * 128:(cc + 1) * 128],
                                     xt, start=st, stop=fin)
                nc.tensor.matmul(ps_g[:], W_hc[hc][:, 384:512], xt,
                                 start=st, stop=fin)
            ps_cc_s[b] = ps_cc
            ps_g_s[b] = ps_g

        def extract(b):
            """Pull QKV(b) out of PSUM (ACT copies + gate exps)."""
            ps_cc, ps_g = ps_cc_s[b], ps_g_s[b]
            qr = [rawp.tile([128, 512], bf16, tag=f"qr{p}", name=f"qr{b}_{p}")
                  for p in range(2)]
            kr = rawp.tile([64, 512], bf16, tag="kr", name=f"kr{b}")
            vr = rawp.tile([64, 512], bf16, tag="vr", name=f"vr{b}")
            for p in range(2):
                nc.scalar.copy(qr[p], ps_cc[p][:])
            nc.scalar.copy(kr, ps_cc[2][0:64, :])
            nc.scalar.copy(vr, ps_cc[2][64:128, :])
            # exp(-gate) per head, packed at rows 0/32/64/96 (legal SBUF
            # partition starts) of one tile; PSUM row slices are exempt from
            # the partition-start rule so ps_g can be read per-head.
            sig_q = smal.tile([128, 512], f32, tag="sig", bufs=2,
                              name=f"eg{b}")
            for p in range(2):
                for hh in range(2):
                    r = 64 * p + 32 * hh
                    nc.scalar.activation(sig_q[r:r + 1, :],
                                         ps_g[r:r + 1, :],
                                         AF.Exp, scale=-1.0)
            qr_s[b], kr_s[b], vr_s[b], sig_s[b] = qr, kr, vr, sig_q

        rms_st = {}

        def rms_phase1(b):
            """RMS-scale matmuls + Newton seeds + k-side chain.  Emitted
            before QKV(b+1) so the Newton/RoPE chains (phase 2) can run on
            DVE/gpsimd while the PE crunches the next projection."""
            qr, kr = qr_s[b], kr_s[b]

            sqs = [None, None]
            pss = [None, None]
            y0s = [None, None]
            for p in range(2):
                sq = sqp.tile([128, 512], bf16, tag=f"sq{p}", name=f"sq{b}_{p}")
                nc.vector.tensor_mul(sq, qr[p], qr[p])
                sqs[p] = sq
            ksq = sqp.tile([64, 512], bf16, tag="ksq", name=f"ksq{b}")
            nc.vector.tensor_mul(ksq, kr, kr)
            for p in range(2):
                ps_rq = psum.tile([128, 512], f32, tag="sc", bufs=2,
                                  name=f"psrq{b}_{p}")
                nc.tensor.matmul(ps_rq[:], esel2, sqs[p], start=True, stop=True)
                pss[p] = ps_rq
            ps_rk = psum.tile([128, 4], f32, tag="sc", bufs=2, name=f"psrk{b}")
            for j in range(4):
                nc.tensor.matmul(ps_rk[:, j:j + 1],
                                 ksq[:, j * 128:(j + 1) * 128],
                                 ones[0:64, :], start=True, stop=True)
            a_t = [None, None]
            for p in range(2):
                y0 = rnp.tile([128, 512], f32, tag=f"rnB{p}", name=f"y0{b}_{p}")
                nc.scalar.activation(y0, pss[p][:].bitcast(u32), AF.Exp,
                                     bias=b_rsq, scale=-0.5 * EXPBIT_SCALE)
                y0s[p] = y0
                if p == 0:
                    a_t[p] = pss[p]
                else:
                    # gpsimd cannot read PSUM; stage the sums in SBUF
                    a_t[p] = rnp.tile([128, 512], f32, tag="rqs",
                                      name=f"rqs{b}")
                    nc.scalar.copy(a_t[p], pss[p][:])

            # k-side Newton ([128,4] ops are ~free) -> rkT ready early for
            # the attention exps
            yk = smal.tile([128, 4], f32, tag="smB", name=f"yk{b}")
            nc.scalar.activation(yk, ps_rk[:].bitcast(u32), AF.Exp,
                                 bias=b_rsq, scale=-0.5 * EXPBIT_SCALE)
            for it in range(2):
                last = (it == 1)
                tk = smal.tile([128, 4], f32, tag="smA", name=f"tk{b}")
                nc.vector.tensor_mul(tk, ps_rk[:], yk)
                nc.vector.tensor_mul(tk, tk, yk)
                nc.vector.tensor_scalar(tk, tk,
                                        (-0.5 * SCALE / HD) if last else (-0.5 / HD),
                                        (1.5 * SCALE) if last else 1.5,
                                        mybir.AluOpType.mult, mybir.AluOpType.add)
                if last:
                    nc.vector.tensor_mul(rkT_sb[:, b * 4:(b + 1) * 4], yk, tk)
                else:
                    ykn = smal.tile([128, 4], f32, tag="smB", name=f"ykn{b}")
                    nc.vector.tensor_mul(ykn, yk, tk)
                    yk = ykn
            rms_st[b] = (a_t, y0s)

        def rope_phase2(b):
            """Newton iterations + RoPE -> qf, kk2.  DVE handles the p=0
            chain and k; gpsimd handles the p=1 Newton and all rotate-half
            copies.  No PE work here — runs concurrently with QKV(b+1)."""
            sp = slice(b * 512, (b + 1) * 512)
            qr, kr = qr_s[b], kr_s[b]
            a_t, y0s = rms_st.pop(b)

            # rotate-half staging (cross-partition copies: DVE only — gpsimd
            # Q7 cores are hardwired to their own 16-partition slice and
            # cannot shuffle across partitions)
            qss = [None, None]
            for p in range(2):
                qs = tmpp.tile([128, 512], bf16, tag=f"qs{p}",
                               name=f"qs{b}_{p}")
                for g in range(2):
                    bb = g * 64
                    nc.vector.tensor_copy(qs[bb:bb + 32, :],
                                          qr[p][bb + 32:bb + 64, :])
                    nc.vector.tensor_copy(qs[bb + 32:bb + 64, :],
                                          qr[p][bb:bb + 32, :])
                qss[p] = qs
            ks = tmpp.tile([64, 512], bf16, tag="ks", name=f"ks{b}")
            nc.vector.tensor_copy(ks[0:32, :], kr[32:64, :])
            nc.vector.tensor_copy(ks[32:64, :], kr[0:32, :])

            rq = [None, None]
            for p in range(2):
                eng = nc.vector if p == 0 else nc.gpsimd
                y0 = y0s[p]
                if p == 0:
                    # z = a*y0 is the only (slow) PSUM read; the bank frees
                    # immediately and z*tn tracks a*y_n in SBUF thereafter
                    z = rnp.tile([128, 512], f32, tag="rnZ", name=f"z{b}")
                    eng.tensor_mul(z, a_t[p][:], y0)
                    src_a = z
                else:
                    src_a = a_t[p]
                for it in range(1):
                    tn = rnp.tile([128, 512], f32, tag=f"rnA{p}",
                                  name=f"tn{b}_{p}")
                    if p == 0:
                        eng.tensor_mul(tn, src_a[:], y0)
                    else:
                        eng.tensor_mul(tn, src_a[:], y0)
                        eng.tensor_mul(tn, tn, y0)
                    if p == 0:
                        pass
                    eng.tensor_scalar(tn, tn, -0.5 / HD, 1.5,
                                      mybir.AluOpType.mult,
                                      mybir.AluOpType.add)
                    yn = rnp.tile([128, 512], f32, tag=f"rnB{p}",
                                  name=f"yn{b}_{p}")
                    eng.tensor_mul(yn, y0, tn)
                    y0 = yn
                rq[p] = y0

            qf = [rawp.tile([128, 512], bf16, tag=f"qf{p}", name=f"qf{b}_{p}",
                            bufs=2) for p in range(2)]

            def rope_q(p):
                t1 = tmpp.tile([128, 512], bf16, tag="t1")
                nc.vector.tensor_mul(t1, qr[p], cosq_sb[:, sp])
                t2 = tmpp.tile([128, 512], bf16, tag="t2")
                nc.vector.tensor_mul(t2, qss[p], sinq_sb[:, sp])
                nc.vector.tensor_add(t2, t1, t2)
                nc.vector.tensor_mul(qf[p], t2, rq[p])

            # p=0 q first (needed by the first scores), then k (needed by
            # every scores), then p=1 q (needed only mid-attention)
            rope_q(0)
            t1k = tmpp.tile([64, 512], bf16, tag="t1")
            nc.vector.tensor_mul(t1k, kr, cosk_sb[:, sp])
            t2k = tmpp.tile([64, 512], bf16, tag="t2")
            nc.vector.tensor_mul(t2k, ks, sink_sb[:, sp])
            nc.vector.tensor_add(kk2[0:64, sp], t1k, t2k)
            nc.vector.tensor_copy(kk2[64:128, sp], kk2[0:64, sp])
            rope_q(1)
            return qf

        def vtransp(b):
            vr = vr_s[b]
            for j in range(4):
                J = b * 4 + j
                ps_v = psum.tile([128, 64], bf16, tag="sc", bufs=2,
                                 name=f"psv{b}_{j}")
                nc.tensor.transpose(ps_v[:], vr[:, j * 128:(j + 1) * 128], id64)
                nc.scalar.copy(v_sb[:, J, 0:64], ps_v[:])

        def attention(b, qf):
            B = b
            sig_q = sig_s[b]
            at = [rawp.tile([128, 512], bf16, tag=f"at{p}", name=f"at{b}_{p}",
                            bufs=2) for p in range(2)]
            attcp = [None, None]
            u_q = smal.tile([128, 512], f32, tag="u", bufs=2, name=f"u{b}")
            nc.vector.memset(u_q, 1.0)

            for p in range(2):
                ps_att = [psum.tile([128, 512], f32, tag="att", bufs=2,
                                    name=f"psatt{b}_{p}_{hh}") for hh in range(2)]
                for J in range(4 * B + 4):
                    off = max(0, (J - 4 * B) * 128)
                    ex = []
                    for hh in range(2):
                        rb = hh * 64
                        ps_s = psum.tile([128, 512], f32, tag="sc", bufs=2,
                                         name="pss")
                        nc.tensor.matmul(
                            ps_s[:, off:512],
                            kk2[rb:rb + 64, J * 128:(J + 1) * 128],
                            qf[p][rb:rb + 64, off:512],
                            start=True, stop=True,
                            tile_position=(rb, 0))
                        et = expp.tile([128, 512], bf16, tag="expT", bufs=5,
                                       name="et")
                        nc.scalar.activation(et[:, off:512], ps_s[:, off:512],
                                             AF.Exp, scale=rkT_sb[:, J:J + 1])
                        if off > 0 or J == 4 * B:
                            nc.vector.tensor_mul(et[:, off:off + 128],
                                                 et[:, off:off + 128], tri)
                        ex.append(et)
                    for hh in range(2):
                        nc.tensor.matmul(
                            ps_att[hh][:, off:512],
                            v_sb[:, J, :],
                            ex[hh][:, off:512],
                            start=(J == 0), stop=(J == 4 * B + 3))

                # drain ps_att immediately: PV values to SBUF (ACT), and the
                # scale chain u=(1+exp(-gate))*den -> s=1/u (DVE, direct PSUM
                # row reads + fast reciprocal).  For p=0 this hides entirely
                # under the p=1 J-loop.
                acp = bcp.tile([128, 512], f32, tag="attcp", name=f"acp{b}_{p}")
                for hh in range(2):
                    r = 64 * p + 32 * hh
                    nc.scalar.copy(acp[64 * hh:64 * hh + 64, :],
                                   ps_att[hh][0:64, :])
                    nc.vector.scalar_tensor_tensor(u_q[r:r + 1, :],
                                                   sig_q[r:r + 1, :], 1.0,
                                                   ps_att[hh][64:65, :],
                                                   mybir.AluOpType.add,
                                                   mybir.AluOpType.mult)
                attcp[p] = acp

            # packed Newton reciprocal: all four denominators in one chain
            s_y = smal.tile([128, 512], f32, tag="sy", bufs=2, name=f"sy{b}")
            nc.scalar.activation(s_y, u_q[:].bitcast(u32), AF.Exp,
                                 bias=b_rcp, scale=-EXPBIT_SCALE)
            for it in range(1):  # seed is ~3.5% -> one iter lands ~1e-3
                tu = smal.tile([128, 512], f32, tag="tu", bufs=2,
                               name=f"tu{b}")
                nc.vector.tensor_mul(tu, u_q, s_y)
                nc.vector.tensor_scalar(tu, tu, -1.0, 2.0,
                                        mybir.AluOpType.mult,
                                        mybir.AluOpType.add)
                s_n = smal.tile([128, 512], f32, tag="sy", bufs=2,
                                name=f"sn{b}")
                nc.vector.tensor_mul(s_n, s_y, tu)
                s_y = s_n

            # broadcast scales to head rows on the PE (two accumulated K=1
            # matmuls) and apply in one [128,512] multiply per head pair
            for p in range(2):
                sbc_ps = psum.tile([128, 512], f32, tag="sc", bufs=2,
                                   name=f"sbc{b}_{p}")
                nc.tensor.matmul(sbc_ps[:], selp[p], s_y, start=True,
                                 stop=True)
                nc.vector.tensor_mul(at[p], attcp[p], sbc_ps[:])
            return at

        def outproj(b, at):
            B = b
            for ss in range(4 * B, 4 * B + 4):
                ls = (ss - 4 * B) * 128
                for qtr in range(4):
                    ps_o = psum.tile([128, 512], f32, tag="qkv3", bufs=3,
                                     name="pso")
                    nc.tensor.matmul(ps_o[:], at[0][:, ls:ls + 128],
                                     Wo_sb[:, 0, qtr * 512:(qtr + 1) * 512],
                                     start=True, stop=False)
                    nc.tensor.matmul(ps_o[:], at[1][:, ls:ls + 128],
                                     Wo_sb[:, 1, qtr * 512:(qtr + 1) * 512],
                                     start=False, stop=True)
                    ot = outs.tile([128, 512], f32, tag="ot")
                    if qtr % 2 == 0:
                        nc.scalar.copy(ot, ps_o[:])
                    else:
                        nc.vector.tensor_copy(ot, ps_o[:])
                    nc.gpsimd.dma_start(
                        out=out_d[ss * 128:(ss + 1) * 128,
                                  qtr * 512:(qtr + 1) * 512],
                        in_=ot)

        # ---------------- pipelined schedule
        load_w_x0()
        make_identity(nc, id64)
        make_upper_triangular(nc, tri, val=1.0, diag=True)
        qkv(0)
        extract(0)
        load_x(1)
        cosq_sb, sinq_sb, cosk_sb, sink_sb = load_tables()
        Wo_sb = big.tile([128, 2, H], bf16, tag="Wo")
        nc.gpsimd.dma_start(out=Wo_sb, in_=Wo_d.ap().rearrange(
            "(cc p) h -> p cc h", p=128))
        at_prev = None
        for B in range(NB):
            if B + 2 < NB:
                load_x(B + 2)
            rms_phase1(B)
            if B + 1 < NB:
                qkv(B + 1)
            vtransp(B)
            qf = rope_phase2(B)
            if at_prev is not None:
                outproj(B - 1, at_prev)
            at_prev = attention(B, qf)
            if B + 1 < NB:
                extract(B + 1)
        outproj(NB - 1, at_prev)

    nc.compile()
    return nc


def _get_nc():
    if "nc" not in _BUILT:
        _BUILT["nc"] = _build_nc()
    return _BUILT["nc"]


# ---------------------------------------------------------------- entry point
def _install_ntff_hook():
    import types
    try:
        import antenv
        if "antenv.axon_hooks" in sys.modules:
            return True
        mod = types.ModuleType("antenv.axon_hooks")
        holder = [None]
        mod.set_axon_ntff_profile_hook = lambda h: holder.__setitem__(0, h)
        mod.get_axon_ntff_profile_hook = lambda: holder[0]
        sys.modules["antenv.axon_hooks"] = mod
        antenv.axon_hooks = mod
        from trn_agent_boot.trn_boot import _ntff_profile_via_ctypes
        hook = _ntff_profile_via_ctypes("/opt/axon/libaxon_pjrt.so")
        if hook is None:
            return False
        mod.set_axon_ntff_profile_hook(hook)
        return True
    except Exception:
        return False


def kernel(hidden_states, Wq, Wk, Wv, Wo, g_q, g_k):
    global LAST_EXEC_NS
    from concourse.bass_utils import run_bass_kernel_spmd

    in_maps = _host_prep(hidden_states, Wq, Wk, Wv, Wo, g_q, g_k)
    nc = _get_nc()
    trace = os.environ.get("KERNEL_TRACE", "0") == "1"
    if trace:
        trace = _install_ntff_hook()
    res = run_bass_kernel_spmd(nc, in_maps, list(range(NCORES)), trace=trace)
    LAST_EXEC_NS = res.exec_time_ns
    out = np.zeros((S, H), np.float32)
    for c in range(NCORES):
        out += res.results[c]["out"]
    return out.reshape(1, S, H).astype(np.float32)
